# revision 1
# baseline (speedup 1.0000x reference)
"""GatedAttention TRN2 kernel — 8-core tensor-parallel (1 kv-head group per core).

Self-contained: host-side shard/layout prep + Bass/Tile kernel + gather.

Per-core dataflow (all device tensors feature-on-partition, "T" layouts):
  qkvT = W_c.T @ xT           (f32r matmuls, PSUM accumulation over 16 h-chunks)
  RMS scales via ones-selector matmuls (partition-dim sums), ln/exp on ACT
  RoPE on DVE with host-prefolded cos/sin tables (gain + rotate-half sign baked)
  scoresT[sj,si] per head, row-tiled head pairs on the PE array
  exp on ACT with per-partition scale = 0.125 * rsqrt(mean k^2)  (no max-sub:
  |scores*scale| <= 8 by Cauchy-Schwarz after RMS norm)
  P@V with V augmented by a ones column (M=65) -> fused softmax denominators
  out_partial = attnT_scaled.T @ Wo_c ; host sums the 8 partials.
"""
import math
import os
import sys
import numpy as np
import ml_dtypes

BF16 = ml_dtypes.bfloat16

H, NH, KVH, HD = 2048, 32, 8, 64
G = NH // KVH          # 4 q heads per core
S = 2048
EPS = 1e-6
THETA = 1000000.0
SCALE = 1.0 / math.sqrt(HD)
NCORES = 8
HC = H // 128          # 16 h-chunks
NB = S // 512          # 4 si-blocks
NJ = S // 128          # 16 sj-chunks

_BUILT = {}
LAST_EXEC_NS = None


# ---------------------------------------------------------------- host prep
def _host_prep(hidden_states, Wq, Wk, Wv, Wo, g_q, g_k):
    x = np.ascontiguousarray(np.asarray(hidden_states, np.float32).reshape(S, H))
    Wq = np.asarray(Wq, np.float32)
    Wk = np.asarray(Wk, np.float32)
    Wv = np.asarray(Wv, np.float32)
    Wo = np.asarray(Wo, np.float32)
    g_q = np.asarray(g_q, np.float32)
    g_k = np.asarray(g_k, np.float32)

    xT = np.ascontiguousarray(x.T).astype(BF16)

    inv_freq = 1.0 / (THETA ** (np.arange(0, HD, 2, dtype=np.float32) / HD))
    pos = np.arange(S, dtype=np.float32)
    emb = np.concatenate([pos[:, None] * inv_freq[None, :]] * 2, axis=-1)  # [S,64]
    cos = np.cos(emb).T.astype(np.float32)   # [64, S]
    sin = np.sin(emb).T.astype(np.float32)
    sign = np.where(np.arange(HD) < HD // 2, -1.0, 1.0).astype(np.float32)[:, None]
    cosq = np.ascontiguousarray(cos * g_q[:, None])
    sinq = np.ascontiguousarray(sin * sign * np.roll(g_q, -32)[:, None])
    cosk = np.ascontiguousarray(cos * g_k[:, None])
    sink = np.ascontiguousarray(sin * sign * np.roll(g_k, -32)[:, None])

    in_maps = []
    for c in range(NCORES):
        Wq_g = Wq[:, c * (G * HD + G):(c + 1) * (G * HD + G)]
        gpad = np.zeros((H, 64), np.float32)
        for p in range(2):
            for hh in range(2):
                gpad[:, 32 * p + hh] = Wq_g[:, G * HD + 2 * p + hh]
        W_c = np.ascontiguousarray(np.concatenate(
            [Wq_g[:, :G * HD],
             Wk[:, c * HD:(c + 1) * HD],
             Wv[:, c * HD:(c + 1) * HD],
             gpad], axis=1))                                   # [H, 448]
        Wo_c = np.ascontiguousarray(Wo[c * G * HD:(c + 1) * G * HD, :])  # [256,H]
        in_maps.append({"xT": xT, "W": W_c.astype(BF16), "Wo": Wo_c.astype(BF16),
                        "cosq": cosq, "sinq": sinq, "cosk": cosk, "sink": sink})
    return in_maps


# ---------------------------------------------------------------- bass build
def _build_nc():
    import concourse.bass as bass
    import concourse.mybir as mybir
    import concourse.tile as tile
    from concourse import bacc
    from concourse.masks import make_identity, make_upper_triangular

    dt = mybir.dt
    f32 = dt.float32
    bf16 = dt.bfloat16
    AF = mybir.ActivationFunctionType

    nc = bacc.Bacc("TRN2", target_bir_lowering=False, debug=False,
                   num_devices=NCORES)

    xT_d = nc.dram_tensor("xT", [H, S], bf16, kind="ExternalInput")
    W_d = nc.dram_tensor("W", [H, 448], bf16, kind="ExternalInput")
    Wo_d = nc.dram_tensor("Wo", [G * HD, H], bf16, kind="ExternalInput")
    cosq_d = nc.dram_tensor("cosq", [HD, S], f32, kind="ExternalInput")
    sinq_d = nc.dram_tensor("sinq", [HD, S], f32, kind="ExternalInput")
    cosk_d = nc.dram_tensor("cosk", [HD, S], f32, kind="ExternalInput")
    sink_d = nc.dram_tensor("sink", [HD, S], f32, kind="ExternalInput")
    out_d = nc.dram_tensor("out", [S, H], f32, kind="ExternalOutput")

    def bcast_rows(src, reps):
        """src [r, n] -> AP iterating [r, reps, n] (each row repeated reps
        times along the destination partition axis)."""
        return bass.AP(tensor=src.tensor, offset=src.offset,
                       ap=[src.ap[0], [0, reps], src.ap[1]])

    import contextlib
    with tile.TileContext(nc) as tc, contextlib.ExitStack() as ctx:
        const = ctx.enter_context(tc.tile_pool(name="const", bufs=1))
        big = ctx.enter_context(tc.tile_pool(name="big", bufs=1))
        xpool = ctx.enter_context(tc.tile_pool(name="xp", bufs=3))
        rawp = ctx.enter_context(tc.tile_pool(name="raw", bufs=2))
        tmpp = ctx.enter_context(tc.tile_pool(name="tmp", bufs=2))
        sqp = ctx.enter_context(tc.tile_pool(name="sq", bufs=2))
        bcp = ctx.enter_context(tc.tile_pool(name="bc", bufs=2))
        expp = ctx.enter_context(tc.tile_pool(name="expp", bufs=4))
        outs = ctx.enter_context(tc.tile_pool(name="outs", bufs=3))
        smal = ctx.enter_context(tc.tile_pool(name="smal", bufs=2))
        psum = ctx.enter_context(tc.tile_pool(name="ps", bufs=1, space="PSUM"))

        # ---------------- constants
        id64 = const.tile([64, 64], f32, tag="id64")
        make_identity(nc, id64)
        tri = const.tile([128, 128], bf16, tag="tri")
        make_upper_triangular(nc, tri, val=1.0, diag=True)
        ones = const.tile([128, 1], f32, tag="ones")
        nc.vector.memset(ones, 1.0)
        esel = const.tile([128, 2], f32, tag="esel")
        nc.vector.memset(esel, 0.0)
        nc.vector.memset(esel[0:64, 0:1], 1.0)
        nc.vector.memset(esel[64:128, 1:2], 1.0)
        SIGMA = 0.0430
        EXPBIT_SCALE = math.log(2.0) / (1 << 23)
        b_rsq = const.tile([128, 1], f32, tag="brsq")
        nc.vector.memset(b_rsq, 0.5 * math.log(2.0) * (127 + SIGMA + 6))
        b_rcp = const.tile([128, 1], f32, tag="brcp")
        nc.vector.memset(b_rcp, math.log(2.0) * (127 + SIGMA))
        u32 = dt.uint32

        # ---------------- resident weights / tables
        W_sb = big.tile([128, HC, 448], bf16, tag="W")
        nc.sync.dma_start(out=W_sb, in_=W_d.ap().rearrange(
            "(hc p) c -> p hc c", p=128))
        Wo_sb = big.tile([128, 2, H], bf16, tag="Wo")
        nc.sync.dma_start(out=Wo_sb, in_=Wo_d.ap().rearrange(
            "(cc p) h -> p cc h", p=128))

        def pair_table(src_d, tag):
            t = big.tile([128, S], f32, tag=tag, name=tag)
            src = src_d.ap()
            ap2 = bass.AP(tensor=src.tensor, offset=src.offset,
                          ap=[[0, 2]] + list(src.ap))
            nc.sync.dma_start(out=t, in_=ap2)
            return t

        cosq_sb = pair_table(cosq_d, "cosq")
        sinq_sb = pair_table(sinq_d, "sinq")
        cosk_sb = big.tile([64, S], f32, tag="cosk")
        nc.sync.dma_start(out=cosk_sb, in_=cosk_d[:, :])
        sink_sb = big.tile([64, S], f32, tag="sink")
        nc.sync.dma_start(out=sink_sb, in_=sink_d[:, :])

        # ---------------- persistent activations
        kk2 = big.tile([128, S], bf16, tag="kk2")
        v_sb = big.tile([128, NJ, 65], bf16, tag="v")
        nc.vector.memset(v_sb[:, :, 64:65], 1.0)
        rkT_sb = big.tile([128, NJ], f32, tag="rkT")

        for sib in range(NB):
            sp = slice(sib * 512, (sib + 1) * 512)

            # ======== QKV projection for this si-block
            ps_cc = [psum.tile([128, 512], f32, tag="qkv3", bufs=3,
                               name=f"pscc{cc}") for cc in range(3)]
            ps_g = psum.tile([64, 512], f32, tag="gate", bufs=1)
            for hc in range(HC):
                xt = xpool.tile([128, 512], bf16, tag="xt")
                nc.gpsimd.dma_start(out=xt, in_=xT_d[hc * 128:(hc + 1) * 128, sp])
                st = (hc == 0)
                fin = (hc == HC - 1)
                for cc in range(3):
                    nc.tensor.matmul(ps_cc[cc][:],
                                     W_sb[:, hc, cc * 128:(cc + 1) * 128],
                                     xt, start=st, stop=fin)
                nc.tensor.matmul(ps_g[:], W_sb[:, hc, 384:448], xt,
                                 start=st, stop=fin)

            qr = [rawp.tile([128, 512], f32, tag=f"qr{p}", name=f"qr{p}")
                  for p in range(2)]
            kr = rawp.tile([64, 512], f32, tag="kr")
            vr = rawp.tile([64, 512], f32, tag="vr")
            for p in range(2):
                nc.scalar.copy(qr[p], ps_cc[p][:])
            nc.scalar.copy(kr, ps_cc[2][0:64, :])
            nc.scalar.copy(vr, ps_cc[2][64:128, :])
            # exp(-gate); sigmoid folded into the per-B scale reciprocal
            sig_t = []
            for p in range(2):
                eg_t = smal.tile([2, 512], f32, tag="sig", bufs=4, name="eg")
                nc.scalar.activation(eg_t, ps_g[32 * p:32 * p + 2, :], AF.Exp,
                                     scale=-1.0)
                sig_t.append(eg_t)

            # ======== RMS scales
            rqt = [None, None]
            for p in range(2):
                sq = sqp.tile([128, 512], f32, tag="sq")
                nc.vector.tensor_mul(sq, qr[p], qr[p])
                ps_rq = psum.tile([2, 512], f32, tag="sc", bufs=2, name="psrq")
                nc.tensor.matmul(ps_rq[:], esel, sq,
                                 start=True, stop=True)
                y0 = smal.tile([2, 512], f32, tag="smB")
                nc.scalar.activation(y0, ps_rq[:].bitcast(u32), AF.Exp,
                                     bias=b_rsq[0:2, :], scale=-0.5 * EXPBIT_SCALE)
                for it in range(2):
                    tn = smal.tile([2, 512], f32, tag="smA", name="tn")
                    nc.vector.tensor_mul(tn, ps_rq[:], y0)
                    nc.vector.tensor_mul(tn, tn, y0)
                    nc.vector.tensor_scalar(tn, tn, -0.5 / HD, 1.5,
                                            mybir.AluOpType.mult, mybir.AluOpType.add)
                    yn = smal.tile([2, 512], f32, tag="smB", name="yn")
                    nc.vector.tensor_mul(yn, y0, tn)
                    y0 = yn
                rqt[p] = y0

            ksq = sqp.tile([64, 512], f32, tag="ksq")
            nc.vector.tensor_mul(ksq, kr, kr)
            ps_rk = psum.tile([128, 4], f32, tag="sc", bufs=2, name="psrk")
            for j in range(4):
                nc.tensor.matmul(ps_rk[:, j:j + 1],
                                 ksq[:, j * 128:(j + 1) * 128],
                                 ones[0:64, :], start=True, stop=True)
            yk = smal.tile([128, 4], f32, tag="smB", name="yk")
            nc.scalar.activation(yk, ps_rk[:].bitcast(u32), AF.Exp,
                                 bias=b_rsq, scale=-0.5 * EXPBIT_SCALE)
            for it in range(2):
                last = (it == 1)
                tk = smal.tile([128, 4], f32, tag="smA", name="tk")
                nc.vector.tensor_mul(tk, ps_rk[:], yk)
                nc.vector.tensor_mul(tk, tk, yk)
                nc.vector.tensor_scalar(tk, tk,
                                        (-0.5 * SCALE / HD) if last else (-0.5 / HD),
                                        (1.5 * SCALE) if last else 1.5,
                                        mybir.AluOpType.mult, mybir.AluOpType.add)
                ykn = smal.tile([128, 4], f32, tag="smB", name="ykn") if not last else None
                if last:
                    nc.vector.tensor_mul(rkT_sb[:, sib * 4:(sib + 1) * 4], yk, tk)
                else:
                    nc.vector.tensor_mul(ykn, yk, tk)
                    yk = ykn

            # ======== RoPE (+ rq fold for q)
            qf = [rawp.tile([128, 512], bf16, tag=f"qf{p}", name=f"qf{p}", bufs=2)
                  for p in range(2)]
            for p in range(2):
                rqb = bcp.tile([128, 512], f32, tag="rqb")
                nc.sync.dma_start(out=rqb, in_=bcast_rows(rqt[p], 64))
                t1 = tmpp.tile([128, 512], f32, tag="t1")
                nc.vector.tensor_mul(t1, qr[p], cosq_sb[:, sp])
                qs = tmpp.tile([128, 512], f32, tag="qs")
                for g in range(2):
                    b = g * 64
                    nc.vector.tensor_copy(qs[b:b + 32, :], qr[p][b + 32:b + 64, :])
                    nc.vector.tensor_copy(qs[b + 32:b + 64, :], qr[p][b:b + 32, :])
                t2 = tmpp.tile([128, 512], f32, tag="t2")
                nc.vector.tensor_mul(t2, qs, sinq_sb[:, sp])
                nc.vector.tensor_add(t2, t1, t2)
                nc.vector.tensor_mul(qf[p], t2, rqb)

            t1k = tmpp.tile([64, 512], f32, tag="t1")
            nc.vector.tensor_mul(t1k, kr, cosk_sb[:, sp])
            ks = tmpp.tile([64, 512], f32, tag="qs")
            nc.vector.tensor_copy(ks[0:32, :], kr[32:64, :])
            nc.vector.tensor_copy(ks[32:64, :], kr[0:32, :])
            t2k = tmpp.tile([64, 512], f32, tag="t2")
            nc.vector.tensor_mul(t2k, ks, sink_sb[:, sp])
            nc.vector.tensor_add(kk2[0:64, sp], t1k, t2k)
            nc.vector.tensor_copy(kk2[64:128, sp], kk2[0:64, sp])

            # ======== V transpose (token-major, raw)
            for j in range(4):
                J = sib * 4 + j
                ps_v = psum.tile([128, 64], f32, tag="sc", bufs=2, name="psv")
                nc.tensor.transpose(ps_v[:], vr[:, j * 128:(j + 1) * 128], id64)
                nc.scalar.copy(v_sb[:, J, 0:64], ps_v[:])

            # ======== attention for si-block B = sib
            B = sib
            at = [rawp.tile([128, 512], bf16, tag=f"at{p}", name=f"at{p}", bufs=2)
                  for p in range(2)]
            for p in range(2):
                ps_att = [psum.tile([128, 512], f32, tag="att", bufs=2,
                                    name=f"psatt{hh}") for hh in range(2)]
                for J in range(4 * B + 4):
                    off = max(0, (J - 4 * B) * 128)
                    ssp = slice(B * 512 + off, (B + 1) * 512)
                    ex = []
                    for hh in range(2):
                        rb = hh * 64
                        ps_s = psum.tile([128, 512], f32, tag="sc", bufs=2,
                                         name="pss")
                        nc.tensor.matmul(
                            ps_s[:, off:512],
                            kk2[rb:rb + 64, J * 128:(J + 1) * 128],
                            qf[p][rb:rb + 64, off:512],
                            start=True, stop=True,
                            tile_position=(rb, 0))
                        et = expp.tile([128, 512], bf16, tag="expT", bufs=5,
                                       name="et")
                        nc.scalar.activation(et[:, off:512], ps_s[:, off:512],
                                             AF.Exp, scale=rkT_sb[:, J:J + 1])
                        if off > 0 or J == 4 * B:
                            nc.vector.tensor_mul(et[:, off:off + 128],
                                                 et[:, off:off + 128], tri)
                        ex.append(et)
                    for hh in range(2):
                        nc.tensor.matmul(
                            ps_att[hh][0:65, off:512],
                            v_sb[:, J, :],
                            ex[hh][:, off:512],
                            start=(J == 0), stop=(J == 4 * B + 3))

                # denominators -> scale s = sigmoid(gate)/den
                den2 = smal.tile([2, 512], f32, tag="smA")
                for hh in range(2):
                    dh = smal.tile([1, 512], f32, tag="smB")
                    nc.scalar.copy(dh, ps_att[hh][64:65, :])
                    nc.sync.dma_start(out=bass.AP(
                        tensor=den2.tensor, offset=den2[hh:hh + 1, :].offset,
                        ap=den2[hh:hh + 1, :].ap), in_=dh)
                u_t = smal.tile([2, 512], f32, tag="den4")
                nc.vector.scalar_tensor_tensor(u_t, sig_t[p], 1.0, den2,
                                               mybir.AluOpType.add,
                                               mybir.AluOpType.mult)
                s_t = smal.tile([2, 512], f32, tag="smB", name="s_t")
                nc.scalar.activation(s_t, u_t[:].bitcast(u32), AF.Exp,
                                     bias=b_rcp[0:2, :], scale=-EXPBIT_SCALE)
                for it in range(2):
                    tu = smal.tile([2, 512], f32, tag="smA", name="tu")
                    nc.vector.tensor_mul(tu, u_t, s_t)
                    nc.vector.tensor_scalar(tu, tu, -1.0, 2.0,
                                            mybir.AluOpType.mult,
                                            mybir.AluOpType.add)
                    s_n = smal.tile([2, 512], f32, tag="smB", name="s_n")
                    nc.vector.tensor_mul(s_n, s_t, tu)
                    s_t = s_n
                sbc = bcp.tile([128, 512], f32, tag="sbc")
                nc.sync.dma_start(out=sbc, in_=bcast_rows(s_t, 64))
                for hh in range(2):
                    rb = hh * 64
                    nc.vector.tensor_mul(at[p][rb:rb + 64, :],
                                         ps_att[hh][0:64, :], sbc[rb:rb + 64, :])

            # ======== output projection for this block's si-chunks
            for ss in range(4 * B, 4 * B + 4):
                ls = (ss - 4 * B) * 128
                for qtr in range(4):
                    ps_o = psum.tile([128, 512], f32, tag="qkv3", bufs=3,
                                     name="pso")
                    nc.tensor.matmul(ps_o[:], at[0][:, ls:ls + 128],
                                     Wo_sb[:, 0, qtr * 512:(qtr + 1) * 512],
                                     start=True, stop=False)
                    nc.tensor.matmul(ps_o[:], at[1][:, ls:ls + 128],
                                     Wo_sb[:, 1, qtr * 512:(qtr + 1) * 512],
                                     start=False, stop=True)
                    ot = outs.tile([128, 512], f32, tag="ot")
                    if qtr % 2 == 0:
                        nc.scalar.copy(ot, ps_o[:])
                    else:
                        nc.vector.tensor_copy(ot, ps_o[:])
                    nc.gpsimd.dma_start(
                        out=out_d[ss * 128:(ss + 1) * 128, qtr * 512:(qtr + 1) * 512],
                        in_=ot)

    nc.compile()
    return nc


def _get_nc():
    if "nc" not in _BUILT:
        _BUILT["nc"] = _build_nc()
    return _BUILT["nc"]


# ---------------------------------------------------------------- entry point
def _install_ntff_hook():
    import types
    try:
        import antenv
        if "antenv.axon_hooks" in sys.modules:
            return True
        mod = types.ModuleType("antenv.axon_hooks")
        holder = [None]
        mod.set_axon_ntff_profile_hook = lambda h: holder.__setitem__(0, h)
        mod.get_axon_ntff_profile_hook = lambda: holder[0]
        sys.modules["antenv.axon_hooks"] = mod
        antenv.axon_hooks = mod
        from trn_agent_boot.trn_boot import _ntff_profile_via_ctypes
        hook = _ntff_profile_via_ctypes("/opt/axon/libaxon_pjrt.so")
        if hook is None:
            return False
        mod.set_axon_ntff_profile_hook(hook)
        return True
    except Exception:
        return False


def kernel(hidden_states, Wq, Wk, Wv, Wo, g_q, g_k):
    global LAST_EXEC_NS
    from concourse.bass_utils import run_bass_kernel_spmd

    in_maps = _host_prep(hidden_states, Wq, Wk, Wv, Wo, g_q, g_k)
    nc = _get_nc()
    trace = os.environ.get("KERNEL_TRACE", "0") == "1"
    if trace:
        trace = _install_ntff_hook()
    res = run_bass_kernel_spmd(nc, in_maps, list(range(NCORES)), trace=trace)
    LAST_EXEC_NS = res.exec_time_ns
    out = np.zeros((S, H), np.float32)
    for c in range(NCORES):
        out += res.results[c]["out"]
    return out.reshape(1, S, H).astype(np.float32)



# revision 16
# speedup vs baseline: 1.2045x; 1.2045x over previous
"""GatedAttention TRN2 kernel — 8-core tensor-parallel (1 kv-head group per core).

v1 restructure vs baseline: xT resident in SBUF (dense PE stream, HAM-warm),
fast PSUM->bf16 evacuation with downstream math in bf16 DVE 2x modes,
rotate-half copies on GpSimd, V transpose via DMA xbar, gate as separate
1-bank pass, fused [4,512] RMS-q chains, causal masks on GpSimd.

Per-core dataflow (feature-on-partition "T" layouts):
  qkvT = W_c.T @ xT           (bf16 matmuls, PSUM accumulation over 16 h-chunks)
  RMS scales via ones-selector matmuls (partition-dim sums), ln/exp on ACT
  RoPE on DVE with host-prefolded bf16 cos/sin tables
  scoresT[sj,si] per head, row-tiled head pairs on the PE array
  exp on ACT with per-partition scale = 0.125 * rsqrt(mean k^2)  (no max-sub:
  |scores*scale| <= 8 by Cauchy-Schwarz after RMS norm)
  P@V with V augmented by a ones column (M=65) -> fused softmax denominators
  out_partial = attnT_scaled.T @ Wo_c ; host sums the 8 partials.
"""
import math
import os
import sys
import numpy as np
import ml_dtypes

BF16 = ml_dtypes.bfloat16

H, NH, KVH, HD = 2048, 32, 8, 64
G = NH // KVH          # 4 q heads per core
S = 2048
EPS = 1e-6
THETA = 1000000.0
SCALE = 1.0 / math.sqrt(HD)
NCORES = 8
HC = H // 128          # 16 h-chunks
NB = S // 512          # 4 si-blocks
NJ = S // 128          # 16 sj-chunks

_BUILT = {}
LAST_EXEC_NS = None


# ---------------------------------------------------------------- host prep
def _host_prep(hidden_states, Wq, Wk, Wv, Wo, g_q, g_k):
    x = np.ascontiguousarray(np.asarray(hidden_states, np.float32).reshape(S, H))
    Wq = np.asarray(Wq, np.float32)
    Wk = np.asarray(Wk, np.float32)
    Wv = np.asarray(Wv, np.float32)
    Wo = np.asarray(Wo, np.float32)
    g_q = np.asarray(g_q, np.float32)
    g_k = np.asarray(g_k, np.float32)

    xT = np.ascontiguousarray(x.T).astype(BF16)

    inv_freq = 1.0 / (THETA ** (np.arange(0, HD, 2, dtype=np.float32) / HD))
    pos = np.arange(S, dtype=np.float32)
    emb = np.concatenate([pos[:, None] * inv_freq[None, :]] * 2, axis=-1)  # [S,64]
    cos = np.cos(emb).T.astype(np.float32)   # [64, S]
    sin = np.sin(emb).T.astype(np.float32)
    sign = np.where(np.arange(HD) < HD // 2, -1.0, 1.0).astype(np.float32)[:, None]
    cosq1 = cos * g_q[:, None]
    sinq1 = sin * sign * np.roll(g_q, -32)[:, None]
    # duplicate to 128 partitions (2 heads per p-pair)
    cosq = np.ascontiguousarray(np.concatenate([cosq1, cosq1], 0)).astype(BF16)
    sinq = np.ascontiguousarray(np.concatenate([sinq1, sinq1], 0)).astype(BF16)
    cosk = np.ascontiguousarray(cos * g_k[:, None]).astype(BF16)
    sink = np.ascontiguousarray(sin * sign * np.roll(g_k, -32)[:, None]).astype(BF16)

    in_maps = []
    for c in range(NCORES):
        Wq_g = Wq[:, c * (G * HD + G):(c + 1) * (G * HD + G)]
        W_c = np.ascontiguousarray(np.concatenate(
            [Wq_g[:, :G * HD],
             Wk[:, c * HD:(c + 1) * HD],
             Wv[:, c * HD:(c + 1) * HD]], axis=1))              # [H, 384]
        gpad = np.zeros((H, 34), np.float32)
        for p in range(2):
            for hh in range(2):
                gpad[:, 32 * p + hh] = Wq_g[:, G * HD + 2 * p + hh]
        Wg_c = np.ascontiguousarray(gpad)                        # [H, 34]
        Wo_c = np.ascontiguousarray(Wo[c * G * HD:(c + 1) * G * HD, :])  # [256,H]
        in_maps.append({"xT": xT, "W": W_c.astype(BF16), "Wg": Wg_c.astype(BF16),
                        "Wo": Wo_c.astype(BF16),
                        "cosq": cosq, "sinq": sinq, "cosk": cosk, "sink": sink})
    return in_maps


# ---------------------------------------------------------------- bass build
def _build_nc():
    import concourse.bass as bass
    import concourse.mybir as mybir
    import concourse.tile as tile
    from concourse import bacc
    from concourse.masks import make_identity, make_upper_triangular

    dt = mybir.dt
    f32 = dt.float32
    bf16 = dt.bfloat16
    u32 = dt.uint32
    AF = mybir.ActivationFunctionType

    nc = bacc.Bacc("TRN2", target_bir_lowering=False, debug=False,
                   num_devices=NCORES)

    xT_d = nc.dram_tensor("xT", [H, S], bf16, kind="ExternalInput")
    W_d = nc.dram_tensor("W", [H, 384], bf16, kind="ExternalInput")
    Wg_d = nc.dram_tensor("Wg", [H, 34], bf16, kind="ExternalInput")
    Wo_d = nc.dram_tensor("Wo", [G * HD, H], bf16, kind="ExternalInput")
    cosq_d = nc.dram_tensor("cosq", [128, S], bf16, kind="ExternalInput")
    sinq_d = nc.dram_tensor("sinq", [128, S], bf16, kind="ExternalInput")
    cosk_d = nc.dram_tensor("cosk", [HD, S], bf16, kind="ExternalInput")
    sink_d = nc.dram_tensor("sink", [HD, S], bf16, kind="ExternalInput")
    out_d = nc.dram_tensor("out", [S, H], f32, kind="ExternalOutput")

    DBG = os.environ.get("KERNEL_DEBUG", "0") == "1"
    if DBG:
        dbg_kk2 = nc.dram_tensor("dbg_kk2", [128, S], bf16, kind="ExternalOutput")
        dbg_v = nc.dram_tensor("dbg_v", [128, NJ * 65], bf16, kind="ExternalOutput")
        dbg_qf = nc.dram_tensor("dbg_qf", [128, NB, 2, 512], bf16,
                                kind="ExternalOutput")
        dbg_rq = nc.dram_tensor("dbg_rq", [4, NB, 512], f32, kind="ExternalOutput")
        dbg_rkT = nc.dram_tensor("dbg_rkT", [128, NJ], f32, kind="ExternalOutput")
        dbg_eg = nc.dram_tensor("dbg_eg", [2, NB, 2, 512], f32,
                                kind="ExternalOutput")
        dbg_at = nc.dram_tensor("dbg_at", [128, NB, 2, 512], bf16,
                                kind="ExternalOutput")
        dbg_den = nc.dram_tensor("dbg_den", [2, NB, 2, 512], f32,
                                 kind="ExternalOutput")

    def bcast_rows(src, reps):
        """src [r, n] -> AP iterating [r, reps, n] (row-replication)."""
        return bass.AP(tensor=src.tensor, offset=src.offset,
                       ap=[src.ap[0], [0, reps], src.ap[1]])

    import contextlib
    with tile.TileContext(nc) as tc, contextlib.ExitStack() as ctx:
        const = ctx.enter_context(tc.tile_pool(name="const", bufs=1))
        big = ctx.enter_context(tc.tile_pool(name="big", bufs=1))
        evp = ctx.enter_context(tc.tile_pool(name="evp", bufs=3))
        tmpp = ctx.enter_context(tc.tile_pool(name="tmp", bufs=2))
        qfp = ctx.enter_context(tc.tile_pool(name="qfp", bufs=3))
        expp = ctx.enter_context(tc.tile_pool(name="expp", bufs=5))
        outs = ctx.enter_context(tc.tile_pool(name="outs", bufs=4))
        smal = ctx.enter_context(tc.tile_pool(name="smal", bufs=2))
        bcp = ctx.enter_context(tc.tile_pool(name="bc", bufs=2))
        psum = ctx.enter_context(tc.tile_pool(name="ps", bufs=1, space="PSUM"))

        # ---------------- constants
        id64 = const.tile([64, 64], bf16, tag="id64")
        make_identity(nc, id64)
        tri = const.tile([128, 128], bf16, tag="tri")
        make_upper_triangular(nc, tri, val=1.0, diag=True)
        ones = const.tile([128, 1], bf16, tag="ones")
        nc.vector.memset(ones, 1.0)
        esel4 = const.tile([128, 2, 4], bf16, tag="esel4")
        nc.vector.memset(esel4, 0.0)
        for p in range(2):
            nc.vector.memset(esel4[0:64, p, 2 * p:2 * p + 1], 1.0)
            nc.vector.memset(esel4[64:128, p, 2 * p + 1:2 * p + 2], 1.0)
        SIGMA = 0.0430
        EXPBIT_SCALE = math.log(2.0) / (1 << 23)
        b_rsq = const.tile([128, 1], f32, tag="brsq")
        nc.vector.memset(b_rsq, 0.5 * math.log(2.0) * (127 + SIGMA + 6))
        b_rcp = const.tile([128, 1], f32, tag="brcp")
        nc.vector.memset(b_rcp, math.log(2.0) * (127 + SIGMA))

        # ---------------- resident weights / tables
        W_sb = big.tile([128, HC, 384], bf16, tag="W")
        nc.sync.dma_start(out=W_sb, in_=W_d.ap().rearrange(
            "(hc p) c -> p hc c", p=128))
        Wg_sb = big.tile([128, HC, 34], bf16, tag="Wg")
        nc.sync.dma_start(out=Wg_sb, in_=Wg_d.ap().rearrange(
            "(hc p) c -> p hc c", p=128))
        Wo_sb = big.tile([128, 2, H], bf16, tag="Wo")
        nc.sync.dma_start(out=Wo_sb, in_=Wo_d.ap().rearrange(
            "(cc p) h -> p cc h", p=128))
        cosq_sb = big.tile([128, S], bf16, tag="cosq")
        nc.sync.dma_start(out=cosq_sb, in_=cosq_d[:, :])
        sinq_sb = big.tile([128, S], bf16, tag="sinq")
        nc.sync.dma_start(out=sinq_sb, in_=sinq_d[:, :])
        cosk_sb = big.tile([64, S], bf16, tag="cosk")
        nc.sync.dma_start(out=cosk_sb, in_=cosk_d[:, :])
        sink_sb = big.tile([64, S], bf16, tag="sink")
        nc.sync.dma_start(out=sink_sb, in_=sink_d[:, :])

        # resident x (feature-on-partition), 16 chunks
        xsb = big.tile([128, HC, S], bf16, tag="xsb")
        for hc in range(HC):
            nc.gpsimd.dma_start(out=xsb[:, hc, :],
                                in_=xT_d[hc * 128:(hc + 1) * 128, :])

        # ---------------- persistent activations
        kk2 = big.tile([128, S], bf16, tag="kk2")
        v_sb = big.tile([128, NJ, 65], bf16, tag="v")
        nc.vector.memset(v_sb[:, :, 64:65], 1.0)
        rkT_sb = big.tile([128, NJ], f32, tag="rkT")
        eg_sb = big.tile([2, NB, 2, 512], bf16, tag="eg")

        for sib in range(NB):
            B = sib
            sp = slice(sib * 512, (sib + 1) * 512)

            # ======== QKV projection for this si-block (3 PSUM banks)
            ps_cc = [psum.tile([128, 512], f32, tag="qkv", bufs=3,
                               name=f"pscc{cc}") for cc in range(3)]
            for hc in range(HC):
                st = (hc == 0)
                fin = (hc == HC - 1)
                for cc in range(3):
                    nc.tensor.matmul(ps_cc[cc][:],
                                     W_sb[:, hc, cc * 128:(cc + 1) * 128],
                                     xsb[:, hc, sp], start=st, stop=fin)

            # fast evacuation PSUM -> bf16 SBUF
            qc = [evp.tile([128, 512], bf16, tag=f"qc{p}", name=f"qc{p}")
                  for p in range(2)]
            for p in range(2):
                nc.vector.tensor_copy(qc[p], ps_cc[p][:])
            kc = evp.tile([64, 512], bf16, tag="kc")
            nc.vector.tensor_copy(kc, ps_cc[2][0:64, :])
            vc = evp.tile([64, 512], bf16, tag="vc")
            nc.vector.tensor_copy(vc, ps_cc[2][64:128, :])

            # ======== RMS q: fused [4,512] sums for both head pairs
            sq = [tmpp.tile([128, 512], bf16, tag=f"sq{p}", name=f"sq{p}")
                  for p in range(2)]
            for p in range(2):
                nc.vector.tensor_mul(sq[p], qc[p], qc[p])
            ps_rq = psum.tile([4, 512], f32, tag="small", bufs=1, name="psrq")
            for p in range(2):
                nc.tensor.matmul(ps_rq[:], esel4[:, p, :], sq[p],
                                 start=(p == 0), stop=(p == 1))
            rqs = smal.tile([4, 512], f32, tag="smC", name="rqs")
            nc.scalar.copy(rqs, ps_rq[:])
            y0 = smal.tile([4, 512], f32, tag="smB", name="y0q")
            nc.scalar.activation(y0, rqs.bitcast(u32), AF.Exp,
                                 bias=b_rsq[0:4, :], scale=-0.5 * EXPBIT_SCALE)
            for it in range(2):
                last = (it == 1)
                tn = smal.tile([4, 512], f32, tag="smA", name="tn")
                nc.vector.tensor_mul(tn, rqs, y0)
                nc.vector.tensor_mul(tn, tn, y0)
                nc.vector.tensor_scalar(tn, tn, -0.5 / HD, 1.5,
                                        mybir.AluOpType.mult, mybir.AluOpType.add)
                yn = smal.tile([4, 512], f32, tag="smBb" if last else "smB",
                               name="yn")
                nc.vector.tensor_mul(yn, y0, tn)
                y0 = yn
            rq4 = y0                                            # [4,512] f32
            if DBG:
                nc.sync.dma_start(out=dbg_rq[:, B, :], in_=rq4)

            # ======== RMS k: [128,4] per-chunk sums
            ksq = tmpp.tile([64, 512], bf16, tag="ksq")
            nc.vector.tensor_mul(ksq, kc, kc)
            ps_rk = psum.tile([128, 4], f32, tag="small", bufs=1, name="psrk")
            for j in range(4):
                nc.tensor.matmul(ps_rk[:, j:j + 1],
                                 ksq[:, j * 128:(j + 1) * 128],
                                 ones[0:64, :], start=True, stop=True)
            rks = smal.tile([128, 4], f32, tag="smC", name="rks")
            nc.scalar.copy(rks, ps_rk[:])
            yk = smal.tile([128, 4], f32, tag="smB", name="yk")
            nc.scalar.activation(yk, rks.bitcast(u32), AF.Exp,
                                 bias=b_rsq, scale=-0.5 * EXPBIT_SCALE)
            for it in range(2):
                last = (it == 1)
                tk = smal.tile([128, 4], f32, tag="smA", name="tk")
                nc.vector.tensor_mul(tk, rks, yk)
                nc.vector.tensor_mul(tk, tk, yk)
                nc.vector.tensor_scalar(tk, tk,
                                        (-0.5 * SCALE / HD) if last else (-0.5 / HD),
                                        (1.5 * SCALE) if last else 1.5,
                                        mybir.AluOpType.mult, mybir.AluOpType.add)
                if last:
                    nc.vector.tensor_mul(rkT_sb[:, sib * 4:(sib + 1) * 4], yk, tk)
                else:
                    ykn = smal.tile([128, 4], f32, tag="smB", name="ykn")
                    nc.vector.tensor_mul(ykn, yk, tk)
                    yk = ykn

            # ======== gate pass ([4,512] in small bank)
            ps_g4 = psum.tile([34, 512], f32, tag="small", bufs=1, name="psg4")
            for hc in range(HC):
                nc.tensor.matmul(ps_g4[:], Wg_sb[:, hc, :], xsb[:, hc, sp],
                                 start=(hc == 0), stop=(hc == HC - 1))
            for p in range(2):
                nc.scalar.activation(eg_sb[:, B, p, :], ps_g4[32 * p:32 * p + 2, :],
                                     AF.Exp, scale=-1.0)
            if DBG:
                egf = smal.tile([2, 2, 512], f32, tag="egf")
                for p in range(2):
                    nc.vector.tensor_copy(egf[:, p, :], eg_sb[:, B, p, :])
                nc.sync.dma_start(out=dbg_eg[:, B, :, :], in_=egf)

            # ======== RoPE q (bf16)
            qf = [qfp.tile([128, 512], bf16, tag=f"qf{p}", name=f"qf{p}")
                  for p in range(2)]
            for p in range(2):
                rqb = bcp.tile([128, 512], f32, tag="rqb")
                nc.sync.dma_start(out=rqb, in_=bcast_rows(rq4[2 * p:2 * p + 2], 64))
                t1 = tmpp.tile([128, 512], bf16, tag="t1")
                nc.vector.tensor_mul(t1, qc[p], cosq_sb[:, sp])
                qs = tmpp.tile([128, 512], bf16, tag="qs")
                for g in range(2):
                    b = g * 64
                    nc.vector.tensor_copy(qs[b:b + 32, :], qc[p][b + 32:b + 64, :])
                    nc.vector.tensor_copy(qs[b + 32:b + 64, :], qc[p][b:b + 32, :])
                t2 = tmpp.tile([128, 512], bf16, tag="t2")
                nc.vector.tensor_mul(t2, qs, sinq_sb[:, sp])
                nc.vector.tensor_add(t2, t1, t2)
                nc.vector.tensor_mul(qf[p], t2, rqb)
                if DBG:
                    nc.sync.dma_start(out=dbg_qf[:, B, p, :], in_=qf[p])

            # ======== RoPE k (bf16)
            t1k = tmpp.tile([64, 512], bf16, tag="t1")
            nc.vector.tensor_mul(t1k, kc, cosk_sb[:, sp])
            ks = tmpp.tile([64, 512], bf16, tag="qs")
            nc.vector.tensor_copy(ks[0:32, :], kc[32:64, :])
            nc.vector.tensor_copy(ks[32:64, :], kc[0:32, :])
            t2k = tmpp.tile([64, 512], bf16, tag="t2")
            nc.vector.tensor_mul(t2k, ks, sink_sb[:, sp])
            nc.vector.tensor_add(kk2[0:64, sp], t1k, t2k)
            nc.vector.tensor_copy(kk2[64:128, sp], kk2[0:64, sp])

            # ======== V transpose (PE transpose, bf16)
            for j in range(4):
                J = sib * 4 + j
                ps_v = psum.tile([128, 64], bf16, tag="small", bufs=1,
                                 name="psv")
                nc.tensor.transpose(ps_v[:], vc[:, j * 128:(j + 1) * 128], id64)
                nc.vector.tensor_copy(v_sb[:, J, 0:64], ps_v[:])

            # ======== attention for si-block B
            at = [qfp.tile([128, 512], bf16, tag=f"at{p}", name=f"at{p}", bufs=2)
                  for p in range(2)]
            for p in range(2):
                ps_att = [psum.tile([128, 512], f32, tag="att", bufs=2,
                                    name=f"psatt{hh}") for hh in range(2)]
                for J in range(4 * B + 4):
                    off = max(0, (J - 4 * B) * 128)
                    ex = []
                    for hh in range(2):
                        rb = hh * 64
                        ps_s = psum.tile([128, 512], f32, tag="sc", bufs=2,
                                         name="pss")
                        nc.tensor.matmul(
                            ps_s[:, off:512],
                            kk2[rb:rb + 64, J * 128:(J + 1) * 128],
                            qf[p][rb:rb + 64, off:512],
                            start=True, stop=True,
                            tile_position=(rb, 0))
                        et = expp.tile([128, 512], bf16, tag="expT", bufs=5,
                                       name="et")
                        nc.scalar.activation(et[:, off:512], ps_s[:, off:512],
                                             AF.Exp, scale=rkT_sb[:, J:J + 1])
                        if off > 0 or J == 4 * B:
                            nc.gpsimd.tensor_mul(et[:, off:off + 128],
                                                 et[:, off:off + 128], tri)
                        ex.append(et)
                    for hh in range(2):
                        nc.tensor.matmul(
                            ps_att[hh][0:65, off:512],
                            v_sb[:, J, :],
                            ex[hh][:, off:512],
                            start=(J == 0), stop=(J == 4 * B + 3))

                # denominators -> scale s = sigmoid(gate)/den
                den2 = smal.tile([2, 512], f32, tag="smA")
                for hh in range(2):
                    dh = smal.tile([1, 512], f32, tag="smB")
                    nc.scalar.copy(dh, ps_att[hh][64:65, :])
                    nc.sync.dma_start(out=bass.AP(
                        tensor=den2.tensor, offset=den2[hh:hh + 1, :].offset,
                        ap=den2[hh:hh + 1, :].ap), in_=dh)
                if DBG:
                    nc.sync.dma_start(out=dbg_den[:, B, p, :], in_=den2)
                u_t = smal.tile([2, 512], f32, tag="den4")
                nc.vector.scalar_tensor_tensor(u_t, eg_sb[:, B, p, :], 1.0, den2,
                                               mybir.AluOpType.add,
                                               mybir.AluOpType.mult)
                s_t = smal.tile([2, 512], f32, tag="smB", name="s_t")
                nc.scalar.activation(s_t, u_t[:].bitcast(u32), AF.Exp,
                                     bias=b_rcp[0:2, :], scale=-EXPBIT_SCALE)
                for it in range(2):
                    tu = smal.tile([2, 512], f32, tag="smA", name="tu")
                    nc.vector.tensor_mul(tu, u_t, s_t)
                    nc.vector.tensor_scalar(tu, tu, -1.0, 2.0,
                                            mybir.AluOpType.mult,
                                            mybir.AluOpType.add)
                    s_n = smal.tile([2, 512], f32, tag="smB", name="s_n")
                    nc.vector.tensor_mul(s_n, s_t, tu)
                    s_t = s_n
                sbc = bcp.tile([128, 512], f32, tag="sbc")
                nc.sync.dma_start(out=sbc, in_=bcast_rows(s_t, 64))
                for hh in range(2):
                    rb = hh * 64
                    nc.vector.tensor_mul(at[p][rb:rb + 64, :],
                                         ps_att[hh][0:64, :], sbc[rb:rb + 64, :])
                if DBG:
                    nc.sync.dma_start(out=dbg_at[:, B, p, :], in_=at[p])

            # ======== output projection for this block's si-chunks
            for ss in range(4 * B, 4 * B + 4):
                ls = (ss - 4 * B) * 128
                for qtr in range(4):
                    ps_o = psum.tile([128, 512], f32, tag="qkv", bufs=3,
                                     name="pso")
                    nc.tensor.matmul(ps_o[:], at[0][:, ls:ls + 128],
                                     Wo_sb[:, 0, qtr * 512:(qtr + 1) * 512],
                                     start=True, stop=False)
                    nc.tensor.matmul(ps_o[:], at[1][:, ls:ls + 128],
                                     Wo_sb[:, 1, qtr * 512:(qtr + 1) * 512],
                                     start=False, stop=True)
                    ot = outs.tile([128, 512], f32, tag="ot")
                    nc.vector.tensor_copy(ot, ps_o[:])
                    nc.gpsimd.dma_start(
                        out=out_d[ss * 128:(ss + 1) * 128, qtr * 512:(qtr + 1) * 512],
                        in_=ot)

        if DBG:
            nc.sync.dma_start(out=dbg_kk2[:, :], in_=kk2)
            nc.sync.dma_start(out=dbg_v[:, :], in_=v_sb[:, :, :])
            nc.sync.dma_start(out=dbg_rkT[:, :], in_=rkT_sb)

    nc.compile()
    return nc


def _get_nc():
    if "nc" not in _BUILT:
        _BUILT["nc"] = _build_nc()
    return _BUILT["nc"]


# ---------------------------------------------------------------- entry point
def _install_ntff_hook():
    import types
    try:
        import antenv
        if "antenv.axon_hooks" in sys.modules:
            return True
        mod = types.ModuleType("antenv.axon_hooks")
        holder = [None]
        mod.set_axon_ntff_profile_hook = lambda h: holder.__setitem__(0, h)
        mod.get_axon_ntff_profile_hook = lambda: holder[0]
        sys.modules["antenv.axon_hooks"] = mod
        antenv.axon_hooks = mod
        from trn_agent_boot.trn_boot import _ntff_profile_via_ctypes
        hook = _ntff_profile_via_ctypes("/opt/axon/libaxon_pjrt.so")
        if hook is None:
            return False
        mod.set_axon_ntff_profile_hook(hook)
        return True
    except Exception:
        return False


def kernel(hidden_states, Wq, Wk, Wv, Wo, g_q, g_k):
    global LAST_EXEC_NS
    from concourse.bass_utils import run_bass_kernel_spmd

    in_maps = _host_prep(hidden_states, Wq, Wk, Wv, Wo, g_q, g_k)
    nc = _get_nc()
    trace = os.environ.get("KERNEL_TRACE", "0") == "1"
    if trace:
        trace = _install_ntff_hook()
    res = run_bass_kernel_spmd(nc, in_maps, list(range(NCORES)), trace=trace)
    LAST_EXEC_NS = res.exec_time_ns
    out = np.zeros((S, H), np.float32)
    for c in range(NCORES):
        out += res.results[c]["out"]
    return out.reshape(1, S, H).astype(np.float32)


# revision 17
# speedup vs baseline: 1.2978x; 1.0775x over previous
"""GatedAttention TRN2 kernel — 8-core tensor-parallel (1 kv-head group per core).

v1 restructure vs baseline: xT resident in SBUF (dense PE stream, HAM-warm),
fast PSUM->bf16 evacuation with downstream math in bf16 DVE 2x modes,
rotate-half copies on GpSimd, V transpose via DMA xbar, gate as separate
1-bank pass, fused [4,512] RMS-q chains, causal masks on GpSimd.

Per-core dataflow (feature-on-partition "T" layouts):
  qkvT = W_c.T @ xT           (bf16 matmuls, PSUM accumulation over 16 h-chunks)
  RMS scales via ones-selector matmuls (partition-dim sums), ln/exp on ACT
  RoPE on DVE with host-prefolded bf16 cos/sin tables
  scoresT[sj,si] per head, row-tiled head pairs on the PE array
  exp on ACT with per-partition scale = 0.125 * rsqrt(mean k^2)  (no max-sub:
  |scores*scale| <= 8 by Cauchy-Schwarz after RMS norm)
  P@V with V augmented by a ones column (M=65) -> fused softmax denominators
  out_partial = attnT_scaled.T @ Wo_c ; host sums the 8 partials.
"""
import math
import os
import sys
import numpy as np
import ml_dtypes

BF16 = ml_dtypes.bfloat16

H, NH, KVH, HD = 2048, 32, 8, 64
G = NH // KVH          # 4 q heads per core
S = 2048
EPS = 1e-6
THETA = 1000000.0
SCALE = 1.0 / math.sqrt(HD)
NCORES = 8
HC = H // 128          # 16 h-chunks
NB = S // 512          # 4 si-blocks
NJ = S // 128          # 16 sj-chunks

_BUILT = {}
LAST_EXEC_NS = None


# ---------------------------------------------------------------- host prep
def _host_prep(hidden_states, Wq, Wk, Wv, Wo, g_q, g_k):
    x = np.ascontiguousarray(np.asarray(hidden_states, np.float32).reshape(S, H))
    Wq = np.asarray(Wq, np.float32)
    Wk = np.asarray(Wk, np.float32)
    Wv = np.asarray(Wv, np.float32)
    Wo = np.asarray(Wo, np.float32)
    g_q = np.asarray(g_q, np.float32)
    g_k = np.asarray(g_k, np.float32)

    xT = np.ascontiguousarray(x.T).astype(BF16)

    inv_freq = 1.0 / (THETA ** (np.arange(0, HD, 2, dtype=np.float32) / HD))
    pos = np.arange(S, dtype=np.float32)
    emb = np.concatenate([pos[:, None] * inv_freq[None, :]] * 2, axis=-1)  # [S,64]
    cos = np.cos(emb).T.astype(np.float32)   # [64, S]
    sin = np.sin(emb).T.astype(np.float32)
    sign = np.where(np.arange(HD) < HD // 2, -1.0, 1.0).astype(np.float32)[:, None]
    cosq1 = cos * g_q[:, None]
    sinq1 = sin * sign * np.roll(g_q, -32)[:, None]
    # duplicate to 128 partitions (2 heads per p-pair)
    cosq = np.ascontiguousarray(np.concatenate([cosq1, cosq1], 0)).astype(BF16)
    sinq = np.ascontiguousarray(np.concatenate([sinq1, sinq1], 0)).astype(BF16)
    cosk = np.ascontiguousarray(cos * g_k[:, None]).astype(BF16)
    sink = np.ascontiguousarray(sin * sign * np.roll(g_k, -32)[:, None]).astype(BF16)

    in_maps = []
    for c in range(NCORES):
        Wq_g = Wq[:, c * (G * HD + G):(c + 1) * (G * HD + G)]
        W_c = np.ascontiguousarray(np.concatenate(
            [Wq_g[:, :G * HD],
             Wk[:, c * HD:(c + 1) * HD],
             Wv[:, c * HD:(c + 1) * HD]], axis=1))              # [H, 384]
        gpad = np.zeros((H, 34), np.float32)
        for p in range(2):
            for hh in range(2):
                gpad[:, 32 * p + hh] = Wq_g[:, G * HD + 2 * p + hh]
        Wg_c = np.ascontiguousarray(gpad)                        # [H, 34]
        Wo_c = np.ascontiguousarray(Wo[c * G * HD:(c + 1) * G * HD, :])  # [256,H]
        in_maps.append({"xT": xT, "W": W_c.astype(BF16), "Wg": Wg_c.astype(BF16),
                        "Wo": Wo_c.astype(BF16),
                        "cosq": cosq, "sinq": sinq, "cosk": cosk, "sink": sink})
    return in_maps


# ---------------------------------------------------------------- bass build
def _build_nc():
    import concourse.bass as bass
    import concourse.mybir as mybir
    import concourse.tile as tile
    from concourse import bacc
    from concourse.masks import make_identity, make_upper_triangular

    dt = mybir.dt
    f32 = dt.float32
    bf16 = dt.bfloat16
    u32 = dt.uint32
    AF = mybir.ActivationFunctionType

    nc = bacc.Bacc("TRN2", target_bir_lowering=False, debug=False,
                   num_devices=NCORES)

    xT_d = nc.dram_tensor("xT", [H, S], bf16, kind="ExternalInput")
    W_d = nc.dram_tensor("W", [H, 384], bf16, kind="ExternalInput")
    Wg_d = nc.dram_tensor("Wg", [H, 34], bf16, kind="ExternalInput")
    Wo_d = nc.dram_tensor("Wo", [G * HD, H], bf16, kind="ExternalInput")
    cosq_d = nc.dram_tensor("cosq", [128, S], bf16, kind="ExternalInput")
    sinq_d = nc.dram_tensor("sinq", [128, S], bf16, kind="ExternalInput")
    cosk_d = nc.dram_tensor("cosk", [HD, S], bf16, kind="ExternalInput")
    sink_d = nc.dram_tensor("sink", [HD, S], bf16, kind="ExternalInput")
    out_d = nc.dram_tensor("out", [S, H], f32, kind="ExternalOutput")

    DBG = os.environ.get("KERNEL_DEBUG", "0") == "1"
    if DBG:
        dbg_kk2 = nc.dram_tensor("dbg_kk2", [128, S], bf16, kind="ExternalOutput")
        dbg_v = nc.dram_tensor("dbg_v", [128, NJ * 65], bf16, kind="ExternalOutput")
        dbg_qf = nc.dram_tensor("dbg_qf", [128, NB, 2, 512], bf16,
                                kind="ExternalOutput")
        dbg_rq = nc.dram_tensor("dbg_rq", [4, NB, 512], f32, kind="ExternalOutput")
        dbg_rkT = nc.dram_tensor("dbg_rkT", [128, NJ], f32, kind="ExternalOutput")
        dbg_eg = nc.dram_tensor("dbg_eg", [2, NB, 2, 512], f32,
                                kind="ExternalOutput")
        dbg_at = nc.dram_tensor("dbg_at", [128, NB, 2, 512], bf16,
                                kind="ExternalOutput")
        dbg_den = nc.dram_tensor("dbg_den", [2, NB, 2, 512], f32,
                                 kind="ExternalOutput")

    def bcast_rows(src, reps):
        """src [r, n] -> AP iterating [r, reps, n] (row-replication)."""
        return bass.AP(tensor=src.tensor, offset=src.offset,
                       ap=[src.ap[0], [0, reps], src.ap[1]])

    import contextlib
    with tile.TileContext(nc) as tc, contextlib.ExitStack() as ctx:
        const = ctx.enter_context(tc.tile_pool(name="const", bufs=1))
        big = ctx.enter_context(tc.tile_pool(name="big", bufs=1))
        evp = ctx.enter_context(tc.tile_pool(name="evp", bufs=3))
        tmpp = ctx.enter_context(tc.tile_pool(name="tmp", bufs=2))
        qfp = ctx.enter_context(tc.tile_pool(name="qfp", bufs=3))
        expp = ctx.enter_context(tc.tile_pool(name="expp", bufs=5))
        outs = ctx.enter_context(tc.tile_pool(name="outs", bufs=4))
        smal = ctx.enter_context(tc.tile_pool(name="smal", bufs=2))
        bcp = ctx.enter_context(tc.tile_pool(name="bc", bufs=2))
        psum = ctx.enter_context(tc.tile_pool(name="ps", bufs=1, space="PSUM"))

        # ---------------- constants
        id64 = const.tile([64, 64], bf16, tag="id64")
        make_identity(nc, id64)
        tri = const.tile([128, 128], bf16, tag="tri")
        make_upper_triangular(nc, tri, val=1.0, diag=True)
        ones = const.tile([128, 1], bf16, tag="ones")
        nc.vector.memset(ones, 1.0)
        esel4 = const.tile([128, 2, 4], bf16, tag="esel4")
        nc.vector.memset(esel4, 0.0)
        for p in range(2):
            nc.vector.memset(esel4[0:64, p, 2 * p:2 * p + 1], 1.0)
            nc.vector.memset(esel4[64:128, p, 2 * p + 1:2 * p + 2], 1.0)
        SIGMA = 0.0430
        EXPBIT_SCALE = math.log(2.0) / (1 << 23)
        b_rsq = const.tile([128, 1], f32, tag="brsq")
        nc.vector.memset(b_rsq, 0.5 * math.log(2.0) * (127 + SIGMA + 6))
        b_rcp = const.tile([128, 1], f32, tag="brcp")
        nc.vector.memset(b_rcp, math.log(2.0) * (127 + SIGMA))

        # ---------------- resident weights / tables
        W_sb = big.tile([128, HC, 384], bf16, tag="W")
        nc.sync.dma_start(out=W_sb, in_=W_d.ap().rearrange(
            "(hc p) c -> p hc c", p=128))
        Wg_sb = big.tile([128, HC, 34], bf16, tag="Wg")
        nc.sync.dma_start(out=Wg_sb, in_=Wg_d.ap().rearrange(
            "(hc p) c -> p hc c", p=128))
        Wo_sb = big.tile([128, 2, H], bf16, tag="Wo")
        nc.sync.dma_start(out=Wo_sb, in_=Wo_d.ap().rearrange(
            "(cc p) h -> p cc h", p=128))
        cosq_sb = big.tile([128, S], bf16, tag="cosq")
        nc.sync.dma_start(out=cosq_sb, in_=cosq_d[:, :])
        sinq_sb = big.tile([128, S], bf16, tag="sinq")
        nc.sync.dma_start(out=sinq_sb, in_=sinq_d[:, :])
        cosk_sb = big.tile([64, S], bf16, tag="cosk")
        nc.sync.dma_start(out=cosk_sb, in_=cosk_d[:, :])
        sink_sb = big.tile([64, S], bf16, tag="sink")
        nc.sync.dma_start(out=sink_sb, in_=sink_d[:, :])

        # resident x (feature-on-partition), 16 chunks
        xsb = big.tile([128, HC, S], bf16, tag="xsb")
        for hc in range(HC):
            nc.gpsimd.dma_start(out=xsb[:, hc, :],
                                in_=xT_d[hc * 128:(hc + 1) * 128, :])

        # ---------------- persistent activations
        kk2 = big.tile([128, S], bf16, tag="kk2")
        v_sb = big.tile([128, NJ, 65], bf16, tag="v")
        nc.vector.memset(v_sb[:, :, 64:65], 1.0)
        rkT_sb = big.tile([128, NJ], f32, tag="rkT")
        eg_sb = big.tile([2, NB, 2, 512], bf16, tag="eg")

        def emit_qkv(sib):
            B = sib
            sp = slice(sib * 512, (sib + 1) * 512)

            # ======== QKV projection for this si-block (3 PSUM banks)
            ps_cc = [psum.tile([128, 512], f32, tag="qkv", bufs=3,
                               name=f"pscc{cc}") for cc in range(3)]
            for hc in range(HC):
                st = (hc == 0)
                fin = (hc == HC - 1)
                for cc in range(3):
                    nc.tensor.matmul(ps_cc[cc][:],
                                     W_sb[:, hc, cc * 128:(cc + 1) * 128],
                                     xsb[:, hc, sp], start=st, stop=fin)

            # fast evacuation PSUM -> bf16 SBUF
            qc = [evp.tile([128, 512], bf16, tag=f"qc{p}", name=f"qc{p}")
                  for p in range(2)]
            for p in range(2):
                nc.vector.tensor_copy(qc[p], ps_cc[p][:])
            kc = evp.tile([64, 512], bf16, tag="kc")
            nc.vector.tensor_copy(kc, ps_cc[2][0:64, :])
            vc = evp.tile([64, 512], bf16, tag="vc")
            nc.vector.tensor_copy(vc, ps_cc[2][64:128, :])

            # ======== RMS q: fused [4,512] sums for both head pairs
            sq = [tmpp.tile([128, 512], bf16, tag=f"sq{p}", name=f"sq{p}")
                  for p in range(2)]
            for p in range(2):
                nc.vector.tensor_mul(sq[p], qc[p], qc[p])
            ps_rq = psum.tile([4, 512], f32, tag="small", bufs=1, name="psrq")
            for p in range(2):
                nc.tensor.matmul(ps_rq[:], esel4[:, p, :], sq[p],
                                 start=(p == 0), stop=(p == 1))
            rqs = smal.tile([4, 512], f32, tag="smC", name="rqs")
            nc.scalar.copy(rqs, ps_rq[:])
            y0 = smal.tile([4, 512], f32, tag="smB", name="y0q")
            nc.scalar.activation(y0, rqs.bitcast(u32), AF.Exp,
                                 bias=b_rsq[0:4, :], scale=-0.5 * EXPBIT_SCALE)
            for it in range(2):
                last = (it == 1)
                tn = smal.tile([4, 512], f32, tag="smA", name="tn")
                nc.vector.tensor_mul(tn, rqs, y0)
                nc.vector.tensor_mul(tn, tn, y0)
                nc.vector.tensor_scalar(tn, tn, -0.5 / HD, 1.5,
                                        mybir.AluOpType.mult, mybir.AluOpType.add)
                yn = smal.tile([4, 512], f32, tag="smBb" if last else "smB",
                               name="yn")
                nc.vector.tensor_mul(yn, y0, tn)
                y0 = yn
            rq4 = y0                                            # [4,512] f32
            if DBG:
                nc.sync.dma_start(out=dbg_rq[:, B, :], in_=rq4)

            # ======== RMS k: [128,4] per-chunk sums
            ksq = tmpp.tile([64, 512], bf16, tag="ksq")
            nc.vector.tensor_mul(ksq, kc, kc)
            ps_rk = psum.tile([128, 4], f32, tag="small", bufs=1, name="psrk")
            for j in range(4):
                nc.tensor.matmul(ps_rk[:, j:j + 1],
                                 ksq[:, j * 128:(j + 1) * 128],
                                 ones[0:64, :], start=True, stop=True)
            rks = smal.tile([128, 4], f32, tag="smC", name="rks")
            nc.scalar.copy(rks, ps_rk[:])
            yk = smal.tile([128, 4], f32, tag="smB", name="yk")
            nc.scalar.activation(yk, rks.bitcast(u32), AF.Exp,
                                 bias=b_rsq, scale=-0.5 * EXPBIT_SCALE)
            for it in range(2):
                last = (it == 1)
                tk = smal.tile([128, 4], f32, tag="smA", name="tk")
                nc.vector.tensor_mul(tk, rks, yk)
                nc.vector.tensor_mul(tk, tk, yk)
                nc.vector.tensor_scalar(tk, tk,
                                        (-0.5 * SCALE / HD) if last else (-0.5 / HD),
                                        (1.5 * SCALE) if last else 1.5,
                                        mybir.AluOpType.mult, mybir.AluOpType.add)
                if last:
                    nc.vector.tensor_mul(rkT_sb[:, sib * 4:(sib + 1) * 4], yk, tk)
                else:
                    ykn = smal.tile([128, 4], f32, tag="smB", name="ykn")
                    nc.vector.tensor_mul(ykn, yk, tk)
                    yk = ykn

            # ======== gate pass ([4,512] in small bank)
            ps_g4 = psum.tile([34, 512], f32, tag="small", bufs=1, name="psg4")
            for hc in range(HC):
                nc.tensor.matmul(ps_g4[:], Wg_sb[:, hc, :], xsb[:, hc, sp],
                                 start=(hc == 0), stop=(hc == HC - 1))
            for p in range(2):
                nc.scalar.activation(eg_sb[:, B, p, :], ps_g4[32 * p:32 * p + 2, :],
                                     AF.Exp, scale=-1.0)
            if DBG:
                egf = smal.tile([2, 2, 512], f32, tag="egf")
                for p in range(2):
                    nc.vector.tensor_copy(egf[:, p, :], eg_sb[:, B, p, :])
                nc.sync.dma_start(out=dbg_eg[:, B, :, :], in_=egf)

            # ======== RoPE q (bf16)
            qf = [qfp.tile([128, 512], bf16, tag=f"qf{p}", name=f"qf{p}")
                  for p in range(2)]
            for p in range(2):
                rqb = bcp.tile([128, 512], f32, tag="rqb")
                nc.sync.dma_start(out=rqb, in_=bcast_rows(rq4[2 * p:2 * p + 2], 64))
                t1 = tmpp.tile([128, 512], bf16, tag="t1")
                nc.vector.tensor_mul(t1, qc[p], cosq_sb[:, sp])
                qs = tmpp.tile([128, 512], bf16, tag="qs")
                for g in range(2):
                    b = g * 64
                    nc.vector.tensor_copy(qs[b:b + 32, :], qc[p][b + 32:b + 64, :])
                    nc.vector.tensor_copy(qs[b + 32:b + 64, :], qc[p][b:b + 32, :])
                t2 = tmpp.tile([128, 512], bf16, tag="t2")
                nc.vector.tensor_mul(t2, qs, sinq_sb[:, sp])
                nc.vector.tensor_add(t2, t1, t2)
                nc.vector.tensor_mul(qf[p], t2, rqb)
                if DBG:
                    nc.sync.dma_start(out=dbg_qf[:, B, p, :], in_=qf[p])

            # ======== RoPE k (bf16)
            t1k = tmpp.tile([64, 512], bf16, tag="t1")
            nc.vector.tensor_mul(t1k, kc, cosk_sb[:, sp])
            ks = tmpp.tile([64, 512], bf16, tag="qs")
            nc.vector.tensor_copy(ks[0:32, :], kc[32:64, :])
            nc.vector.tensor_copy(ks[32:64, :], kc[0:32, :])
            t2k = tmpp.tile([64, 512], bf16, tag="t2")
            nc.vector.tensor_mul(t2k, ks, sink_sb[:, sp])
            nc.vector.tensor_add(kk2[0:64, sp], t1k, t2k)
            nc.vector.tensor_copy(kk2[64:128, sp], kk2[0:64, sp])

            # ======== V transpose (PE transpose, bf16)
            for j in range(4):
                J = sib * 4 + j
                ps_v = psum.tile([128, 64], bf16, tag="small", bufs=1,
                                 name="psv")
                nc.tensor.transpose(ps_v[:], vc[:, j * 128:(j + 1) * 128], id64)
                nc.vector.tensor_copy(v_sb[:, J, 0:64], ps_v[:])

            return {'qf': qf}

        def emit_att(sib, st):
            B = sib
            qf = st['qf']
            # ======== attention for si-block B
            at = [qfp.tile([128, 512], bf16, tag=f"at{p}", name=f"at{p}", bufs=2)
                  for p in range(2)]
            for p in range(2):
                ps_att = [psum.tile([128, 512], f32, tag="att", bufs=2,
                                    name=f"psatt{hh}") for hh in range(2)]
                for J in range(4 * B + 4):
                    off = max(0, (J - 4 * B) * 128)
                    ex = []
                    for hh in range(2):
                        rb = hh * 64
                        ps_s = psum.tile([128, 512], f32, tag="sc", bufs=2,
                                         name="pss")
                        nc.tensor.matmul(
                            ps_s[:, off:512],
                            kk2[rb:rb + 64, J * 128:(J + 1) * 128],
                            qf[p][rb:rb + 64, off:512],
                            start=True, stop=True,
                            tile_position=(rb, 0))
                        et = expp.tile([128, 512], bf16, tag="expT", bufs=5,
                                       name="et")
                        nc.scalar.activation(et[:, off:512], ps_s[:, off:512],
                                             AF.Exp, scale=rkT_sb[:, J:J + 1])
                        if off > 0 or J == 4 * B:
                            nc.gpsimd.tensor_mul(et[:, off:off + 128],
                                                 et[:, off:off + 128], tri)
                        ex.append(et)
                    for hh in range(2):
                        nc.tensor.matmul(
                            ps_att[hh][0:65, off:512],
                            v_sb[:, J, :],
                            ex[hh][:, off:512],
                            start=(J == 0), stop=(J == 4 * B + 3))

                # denominators -> scale s = sigmoid(gate)/den
                den2 = smal.tile([2, 512], f32, tag="smA")
                for hh in range(2):
                    dh = smal.tile([1, 512], f32, tag="smB")
                    nc.scalar.copy(dh, ps_att[hh][64:65, :])
                    nc.sync.dma_start(out=bass.AP(
                        tensor=den2.tensor, offset=den2[hh:hh + 1, :].offset,
                        ap=den2[hh:hh + 1, :].ap), in_=dh)
                if DBG:
                    nc.sync.dma_start(out=dbg_den[:, B, p, :], in_=den2)
                u_t = smal.tile([2, 512], f32, tag="den4")
                nc.vector.scalar_tensor_tensor(u_t, eg_sb[:, B, p, :], 1.0, den2,
                                               mybir.AluOpType.add,
                                               mybir.AluOpType.mult)
                s_t = smal.tile([2, 512], f32, tag="smB", name="s_t")
                nc.scalar.activation(s_t, u_t[:].bitcast(u32), AF.Exp,
                                     bias=b_rcp[0:2, :], scale=-EXPBIT_SCALE)
                for it in range(2):
                    tu = smal.tile([2, 512], f32, tag="smA", name="tu")
                    nc.vector.tensor_mul(tu, u_t, s_t)
                    nc.vector.tensor_scalar(tu, tu, -1.0, 2.0,
                                            mybir.AluOpType.mult,
                                            mybir.AluOpType.add)
                    s_n = smal.tile([2, 512], f32, tag="smB", name="s_n")
                    nc.vector.tensor_mul(s_n, s_t, tu)
                    s_t = s_n
                sbc = bcp.tile([128, 512], f32, tag="sbc")
                nc.sync.dma_start(out=sbc, in_=bcast_rows(s_t, 64))
                for hh in range(2):
                    rb = hh * 64
                    nc.vector.tensor_mul(at[p][rb:rb + 64, :],
                                         ps_att[hh][0:64, :], sbc[rb:rb + 64, :])
                if DBG:
                    nc.sync.dma_start(out=dbg_at[:, B, p, :], in_=at[p])

            # ======== output projection for this block's si-chunks
            for ss in range(4 * B, 4 * B + 4):
                ls = (ss - 4 * B) * 128
                for qtr in range(4):
                    ps_o = psum.tile([128, 512], f32, tag="qkv", bufs=3,
                                     name="pso")
                    nc.tensor.matmul(ps_o[:], at[0][:, ls:ls + 128],
                                     Wo_sb[:, 0, qtr * 512:(qtr + 1) * 512],
                                     start=True, stop=False)
                    nc.tensor.matmul(ps_o[:], at[1][:, ls:ls + 128],
                                     Wo_sb[:, 1, qtr * 512:(qtr + 1) * 512],
                                     start=False, stop=True)
                    ot = outs.tile([128, 512], f32, tag="ot")
                    nc.vector.tensor_copy(ot, ps_o[:])
                    nc.gpsimd.dma_start(
                        out=out_d[ss * 128:(ss + 1) * 128, qtr * 512:(qtr + 1) * 512],
                        in_=ot)

        st = {}
        st[0] = emit_qkv(0)
        for sib in range(NB):
            if sib + 1 < NB:
                st[sib + 1] = emit_qkv(sib + 1)
            emit_att(sib, st[sib])
            del st[sib]

        if DBG:
            nc.sync.dma_start(out=dbg_kk2[:, :], in_=kk2)
            nc.sync.dma_start(out=dbg_v[:, :], in_=v_sb[:, :, :])
            nc.sync.dma_start(out=dbg_rkT[:, :], in_=rkT_sb)

    nc.compile()
    return nc


def _get_nc():
    if "nc" not in _BUILT:
        _BUILT["nc"] = _build_nc()
    return _BUILT["nc"]


# ---------------------------------------------------------------- entry point
def _install_ntff_hook():
    import types
    try:
        import antenv
        if "antenv.axon_hooks" in sys.modules:
            return True
        mod = types.ModuleType("antenv.axon_hooks")
        holder = [None]
        mod.set_axon_ntff_profile_hook = lambda h: holder.__setitem__(0, h)
        mod.get_axon_ntff_profile_hook = lambda: holder[0]
        sys.modules["antenv.axon_hooks"] = mod
        antenv.axon_hooks = mod
        from trn_agent_boot.trn_boot import _ntff_profile_via_ctypes
        hook = _ntff_profile_via_ctypes("/opt/axon/libaxon_pjrt.so")
        if hook is None:
            return False
        mod.set_axon_ntff_profile_hook(hook)
        return True
    except Exception:
        return False


def kernel(hidden_states, Wq, Wk, Wv, Wo, g_q, g_k):
    global LAST_EXEC_NS
    from concourse.bass_utils import run_bass_kernel_spmd

    in_maps = _host_prep(hidden_states, Wq, Wk, Wv, Wo, g_q, g_k)
    nc = _get_nc()
    trace = os.environ.get("KERNEL_TRACE", "0") == "1"
    if trace:
        trace = _install_ntff_hook()
    res = run_bass_kernel_spmd(nc, in_maps, list(range(NCORES)), trace=trace)
    LAST_EXEC_NS = res.exec_time_ns
    out = np.zeros((S, H), np.float32)
    for c in range(NCORES):
        out += res.results[c]["out"]
    return out.reshape(1, S, H).astype(np.float32)


# revision 25
# speedup vs baseline: 2.0050x; 1.5449x over previous
"""GatedAttention TRN2 kernel — 8-core tensor-parallel (1 kv-head group per core).

v1 restructure vs baseline: xT resident in SBUF (dense PE stream, HAM-warm),
fast PSUM->bf16 evacuation with downstream math in bf16 DVE 2x modes,
rotate-half copies on GpSimd, V transpose via DMA xbar, gate as separate
1-bank pass, fused [4,512] RMS-q chains, causal masks on GpSimd.

Per-core dataflow (feature-on-partition "T" layouts):
  qkvT = W_c.T @ xT           (bf16 matmuls, PSUM accumulation over 16 h-chunks)
  RMS scales via ones-selector matmuls (partition-dim sums), ln/exp on ACT
  RoPE on DVE with host-prefolded bf16 cos/sin tables
  scoresT[sj,si] per head, row-tiled head pairs on the PE array
  exp on ACT with per-partition scale = 0.125 * rsqrt(mean k^2)  (no max-sub:
  |scores*scale| <= 8 by Cauchy-Schwarz after RMS norm)
  P@V with V augmented by a ones column (M=65) -> fused softmax denominators
  out_partial = attnT_scaled.T @ Wo_c ; host sums the 8 partials.
"""
import math
import os
import sys
import numpy as np
import ml_dtypes

BF16 = ml_dtypes.bfloat16

H, NH, KVH, HD = 2048, 32, 8, 64
G = NH // KVH          # 4 q heads per core
S = 2048
EPS = 1e-6
THETA = 1000000.0
SCALE = 1.0 / math.sqrt(HD)
NCORES = 8
HC = H // 128          # 16 h-chunks
NB = S // 512          # 4 si-blocks
NJ = S // 128          # 16 sj-chunks

_BUILT = {}
LAST_EXEC_NS = None


# ---------------------------------------------------------------- host prep
def _host_prep(hidden_states, Wq, Wk, Wv, Wo, g_q, g_k):
    x = np.ascontiguousarray(np.asarray(hidden_states, np.float32).reshape(S, H))
    Wq = np.asarray(Wq, np.float32)
    Wk = np.asarray(Wk, np.float32)
    Wv = np.asarray(Wv, np.float32)
    Wo = np.asarray(Wo, np.float32)
    g_q = np.asarray(g_q, np.float32)
    g_k = np.asarray(g_k, np.float32)

    xT = np.ascontiguousarray(x.T).astype(BF16)

    inv_freq = 1.0 / (THETA ** (np.arange(0, HD, 2, dtype=np.float32) / HD))
    pos = np.arange(S, dtype=np.float32)
    emb = np.concatenate([pos[:, None] * inv_freq[None, :]] * 2, axis=-1)  # [S,64]
    cos = np.cos(emb).T.astype(np.float32)   # [64, S]
    sin = np.sin(emb).T.astype(np.float32)
    sign = np.where(np.arange(HD) < HD // 2, -1.0, 1.0).astype(np.float32)[:, None]
    cosq1 = cos * g_q[:, None]
    sinq1 = sin * sign * np.roll(g_q, -32)[:, None]
    # duplicate to 128 partitions (2 heads per p-pair)
    cosq = np.ascontiguousarray(np.concatenate([cosq1, cosq1], 0)).astype(BF16)
    sinq = np.ascontiguousarray(np.concatenate([sinq1, sinq1], 0)).astype(BF16)
    cosk = np.ascontiguousarray(cos * g_k[:, None]).astype(BF16)
    sink = np.ascontiguousarray(sin * sign * np.roll(g_k, -32)[:, None]).astype(BF16)

    in_maps = []
    for c in range(NCORES):
        Wq_g = Wq[:, c * (G * HD + G):(c + 1) * (G * HD + G)]
        W_c = np.ascontiguousarray(np.concatenate(
            [Wq_g[:, :G * HD],
             Wk[:, c * HD:(c + 1) * HD],
             Wv[:, c * HD:(c + 1) * HD]], axis=1))              # [H, 384]
        gpad = np.zeros((H, 34), np.float32)
        for p in range(2):
            for hh in range(2):
                gpad[:, 32 * p + hh] = Wq_g[:, G * HD + 2 * p + hh]
        Wg_c = np.ascontiguousarray(gpad)                        # [H, 34]
        Wo_c = np.ascontiguousarray(Wo[c * G * HD:(c + 1) * G * HD, :])  # [256,H]
        sel2 = np.zeros((34, 128), np.float32)
        for bp in (0, 32):
            sel2[bp, 0:64] = 1.0
            sel2[bp + 1, 64:128] = 1.0
        in_maps.append({"xT": xT, "W": W_c.astype(BF16), "Wg": Wg_c.astype(BF16),
                        "Wo": Wo_c.astype(BF16), "sel2": sel2,
                        "cosq": cosq, "sinq": sinq, "cosk": cosk, "sink": sink})
    return in_maps


# ---------------------------------------------------------------- bass build
def _build_nc():
    import concourse.bass as bass
    import concourse.mybir as mybir
    import concourse.tile as tile
    from concourse import bacc
    from concourse.masks import make_identity, make_upper_triangular

    dt = mybir.dt
    f32 = dt.float32
    bf16 = dt.bfloat16
    u32 = dt.uint32
    AF = mybir.ActivationFunctionType

    nc = bacc.Bacc("TRN2", target_bir_lowering=False, debug=False,
                   num_devices=NCORES)

    xT_d = nc.dram_tensor("xT", [H, S], bf16, kind="ExternalInput")
    W_d = nc.dram_tensor("W", [H, 384], bf16, kind="ExternalInput")
    Wg_d = nc.dram_tensor("Wg", [H, 34], bf16, kind="ExternalInput")
    Wo_d = nc.dram_tensor("Wo", [G * HD, H], bf16, kind="ExternalInput")
    cosq_d = nc.dram_tensor("cosq", [128, S], bf16, kind="ExternalInput")
    sinq_d = nc.dram_tensor("sinq", [128, S], bf16, kind="ExternalInput")
    cosk_d = nc.dram_tensor("cosk", [HD, S], bf16, kind="ExternalInput")
    sink_d = nc.dram_tensor("sink", [HD, S], bf16, kind="ExternalInput")
    sel2_d = nc.dram_tensor("sel2", [34, 128], f32, kind="ExternalInput")
    out_d = nc.dram_tensor("out", [S, H], f32, kind="ExternalOutput")

    DBG = os.environ.get("KERNEL_DEBUG", "0") == "1"
    if DBG:
        dbg_kk2 = nc.dram_tensor("dbg_kk2", [128, S], bf16, kind="ExternalOutput")
        dbg_v = nc.dram_tensor("dbg_v", [128, NJ * 65], bf16, kind="ExternalOutput")
        dbg_qf = nc.dram_tensor("dbg_qf", [128, NB, 2, 512], bf16,
                                kind="ExternalOutput")
        dbg_rq = nc.dram_tensor("dbg_rq", [4, NB, 512], f32, kind="ExternalOutput")
        dbg_rkT = nc.dram_tensor("dbg_rkT", [128, NJ], f32, kind="ExternalOutput")
        dbg_eg = nc.dram_tensor("dbg_eg", [2, NB, 2, 512], f32,
                                kind="ExternalOutput")
        dbg_at = nc.dram_tensor("dbg_at", [128, NB, 2, 512], bf16,
                                kind="ExternalOutput")
        dbg_den = nc.dram_tensor("dbg_den", [2, NB, 2, 512], f32,
                                 kind="ExternalOutput")

    def bcast_rows(src, reps):
        """src [r, n] -> AP iterating [r, reps, n] (row-replication)."""
        return bass.AP(tensor=src.tensor, offset=src.offset,
                       ap=[src.ap[0], [0, reps], src.ap[1]])

    import contextlib
    with tile.TileContext(nc) as tc, contextlib.ExitStack() as ctx:
        const = ctx.enter_context(tc.tile_pool(name="const", bufs=1))
        big = ctx.enter_context(tc.tile_pool(name="big", bufs=1))
        evp = ctx.enter_context(tc.tile_pool(name="evp", bufs=3))
        tmpp = ctx.enter_context(tc.tile_pool(name="tmp", bufs=2))
        qfp = ctx.enter_context(tc.tile_pool(name="qfp", bufs=3))
        expp = ctx.enter_context(tc.tile_pool(name="expp", bufs=5))
        outs = ctx.enter_context(tc.tile_pool(name="outs", bufs=4))
        smal = ctx.enter_context(tc.tile_pool(name="smal", bufs=2))
        bcp = ctx.enter_context(tc.tile_pool(name="bc", bufs=2))
        psum = ctx.enter_context(tc.tile_pool(name="ps", bufs=1, space="PSUM"))

        # ---------------- constants
        id64 = const.tile([64, 64], bf16, tag="id64")
        make_identity(nc, id64)
        tri = const.tile([128, 128], bf16, tag="tri")
        make_upper_triangular(nc, tri, val=1.0, diag=True)
        ones = const.tile([128, 1], bf16, tag="ones")
        nc.vector.memset(ones, 1.0)
        esel4 = const.tile([128, 2, 34], bf16, tag="esel4")
        nc.vector.memset(esel4, 0.0)
        for p in range(2):
            nc.vector.memset(esel4[0:64, p, 32 * p:32 * p + 1], 1.0)
            nc.vector.memset(esel4[64:128, p, 32 * p + 1:32 * p + 2], 1.0)
        sel2 = const.tile([34, 128], f32, tag="sel2")
        nc.sync.dma_start(out=sel2, in_=sel2_d[:, :])
        SIGMA = 0.0430
        EXPBIT_SCALE = math.log(2.0) / (1 << 23)
        b_rsq = const.tile([128, 1], f32, tag="brsq")
        nc.vector.memset(b_rsq, 0.5 * math.log(2.0) * (127 + SIGMA + 6))
        b_rcp = const.tile([128, 1], f32, tag="brcp")
        nc.vector.memset(b_rcp, math.log(2.0) * (127 + SIGMA))

        # ---------------- resident weights / tables
        W_sb = big.tile([128, HC, 384], bf16, tag="W")
        nc.sync.dma_start(out=W_sb, in_=W_d.ap().rearrange(
            "(hc p) c -> p hc c", p=128))
        Wg_sb = big.tile([128, HC, 34], bf16, tag="Wg")
        nc.sync.dma_start(out=Wg_sb, in_=Wg_d.ap().rearrange(
            "(hc p) c -> p hc c", p=128))
        Wo_sb = big.tile([128, 2, H], bf16, tag="Wo")
        nc.sync.dma_start(out=Wo_sb, in_=Wo_d.ap().rearrange(
            "(cc p) h -> p cc h", p=128))
        cosq_sb = big.tile([128, S], bf16, tag="cosq")
        nc.sync.dma_start(out=cosq_sb, in_=cosq_d[:, :])
        sinq_sb = big.tile([128, S], bf16, tag="sinq")
        nc.sync.dma_start(out=sinq_sb, in_=sinq_d[:, :])
        cosk_sb = big.tile([64, S], bf16, tag="cosk")
        nc.sync.dma_start(out=cosk_sb, in_=cosk_d[:, :])
        sink_sb = big.tile([64, S], bf16, tag="sink")
        nc.sync.dma_start(out=sink_sb, in_=sink_d[:, :])

        # resident x (feature-on-partition), 16 chunks
        xsb = big.tile([128, HC, S], bf16, tag="xsb")
        for hc in range(HC):
            nc.gpsimd.dma_start(out=xsb[:, hc, :],
                                in_=xT_d[hc * 128:(hc + 1) * 128, :])

        # ---------------- persistent activations
        kk2 = big.tile([128, S], bf16, tag="kk2")
        v_sb = big.tile([128, NJ, 65], bf16, tag="v")
        nc.vector.memset(v_sb[:, :, 64:65], 1.0)
        rkT_sb = big.tile([128, NJ], f32, tag="rkT")
        eg_sb = big.tile([2, NB, 2, 512], bf16, tag="eg")

        def emit_qkv(sib):
            B = sib
            sp = slice(sib * 512, (sib + 1) * 512)

            # ======== QKV projection for this si-block (3 PSUM banks)
            ps_cc = [psum.tile([128, 512], f32, tag="qkv", bufs=3,
                               name=f"pscc{cc}") for cc in range(3)]
            for hc in range(HC):
                st = (hc == 0)
                fin = (hc == HC - 1)
                for cc in range(3):
                    nc.tensor.matmul(ps_cc[cc][:],
                                     W_sb[:, hc, cc * 128:(cc + 1) * 128],
                                     xsb[:, hc, sp], start=st, stop=fin)

            # fast evacuation PSUM -> bf16 SBUF
            qc = [evp.tile([128, 512], bf16, tag=f"qc{p}", name=f"qc{p}")
                  for p in range(2)]
            for p in range(2):
                nc.vector.tensor_copy(qc[p], ps_cc[p][:])
            kc = evp.tile([64, 512], bf16, tag="kc")
            nc.vector.tensor_copy(kc, ps_cc[2][0:64, :])
            vc = evp.tile([64, 512], bf16, tag="vc")
            nc.vector.tensor_copy(vc, ps_cc[2][64:128, :])

            # ======== RMS q: fused [4,512] sums for both head pairs
            sq = [tmpp.tile([128, 512], bf16, tag=f"sq{p}", name=f"sq{p}")
                  for p in range(2)]
            for p in range(2):
                nc.vector.tensor_mul(sq[p], qc[p], qc[p])
            ps_rq = psum.tile([34, 512], f32, tag="small", bufs=1, name="psrq")
            for p in range(2):
                nc.tensor.matmul(ps_rq[:], esel4[:, p, :], sq[p],
                                 start=(p == 0), stop=(p == 1))
            rqs = smal.tile([34, 512], f32, tag="smC", name="rqs")
            nc.scalar.copy(rqs, ps_rq[:])
            y0 = smal.tile([34, 512], f32, tag="smB", name="y0q")
            nc.scalar.activation(y0, rqs.bitcast(u32), AF.Exp,
                                 bias=b_rsq[0:34, :], scale=-0.5 * EXPBIT_SCALE)
            for it in range(2):
                last = (it == 1)
                tn = smal.tile([34, 512], f32, tag="smA", name="tn")
                nc.vector.tensor_mul(tn, rqs, y0)
                nc.vector.tensor_mul(tn, tn, y0)
                nc.vector.tensor_scalar(tn, tn, -0.5 / HD, 1.5,
                                        mybir.AluOpType.mult, mybir.AluOpType.add)
                if last:
                    yn = smal.tile([34, 512], f32, tag="smBb", bufs=3, name="yn")
                else:
                    yn = smal.tile([34, 512], f32, tag="smB", name="yn")
                nc.vector.tensor_mul(yn, y0, tn)
                y0 = yn
            rq4 = y0                                            # [34,512] f32
            if DBG:
                for p in range(2):
                    nc.sync.dma_start(out=dbg_rq[2 * p:2 * p + 2, B, :],
                                      in_=rq4[32 * p:32 * p + 2, :])

            # ======== RMS k: [128,4] per-chunk sums
            ksq = tmpp.tile([64, 512], bf16, tag="ksq")
            nc.vector.tensor_mul(ksq, kc, kc)
            ps_rk = psum.tile([128, 4], f32, tag="small", bufs=1, name="psrk")
            for j in range(4):
                nc.tensor.matmul(ps_rk[:, j:j + 1],
                                 ksq[:, j * 128:(j + 1) * 128],
                                 ones[0:64, :], start=True, stop=True)
            rks = smal.tile([128, 4], f32, tag="smC", name="rks")
            nc.scalar.copy(rks, ps_rk[:])
            yk = smal.tile([128, 4], f32, tag="smB", name="yk")
            nc.scalar.activation(yk, rks.bitcast(u32), AF.Exp,
                                 bias=b_rsq, scale=-0.5 * EXPBIT_SCALE)
            for it in range(2):
                last = (it == 1)
                tk = smal.tile([128, 4], f32, tag="smA", name="tk")
                nc.vector.tensor_mul(tk, rks, yk)
                nc.vector.tensor_mul(tk, tk, yk)
                nc.vector.tensor_scalar(tk, tk,
                                        (-0.5 * SCALE / HD) if last else (-0.5 / HD),
                                        (1.5 * SCALE) if last else 1.5,
                                        mybir.AluOpType.mult, mybir.AluOpType.add)
                if last:
                    nc.vector.tensor_mul(rkT_sb[:, sib * 4:(sib + 1) * 4], yk, tk)
                else:
                    ykn = smal.tile([128, 4], f32, tag="smB", name="ykn")
                    nc.vector.tensor_mul(ykn, yk, tk)
                    yk = ykn

            # ======== V transpose (PE transpose, bf16)
            for j in range(4):
                J = sib * 4 + j
                ps_v = psum.tile([128, 64], bf16, tag="small", bufs=1,
                                 name="psv")
                nc.tensor.transpose(ps_v[:], vc[:, j * 128:(j + 1) * 128], id64)
                nc.vector.tensor_copy(v_sb[:, J, 0:64], ps_v[:])

            return {'qc': qc, 'kc': kc, 'rq4': rq4}

        def emit_att(sib, st):
            B = sib
            qc, kc, rq4 = st['qc'], st['kc'], st['rq4']
            sp = slice(sib * 512, (sib + 1) * 512)
            # ======== RoPE q (bf16)
            qf = [qfp.tile([128, 512], bf16, tag=f"qf{p}", name=f"qf{p}")
                  for p in range(2)]
            for p in range(2):
                ps_rqb = psum.tile([128, 512], f32, tag="small", bufs=1,
                                   name="psrqb")
                nc.tensor.matmul(ps_rqb[:], sel2[32 * p:32 * p + 2, :],
                                 rq4[32 * p:32 * p + 2, :],
                                 start=True, stop=True,
                                 tile_position=(32 * p, 0))
                t1 = tmpp.tile([128, 512], bf16, tag="t1")
                nc.vector.tensor_mul(t1, qc[p], cosq_sb[:, sp])
                qs = tmpp.tile([128, 512], bf16, tag="qs")
                for g in range(2):
                    b = g * 64
                    nc.vector.tensor_copy(qs[b:b + 32, :], qc[p][b + 32:b + 64, :])
                    nc.vector.tensor_copy(qs[b + 32:b + 64, :], qc[p][b:b + 32, :])
                t2 = tmpp.tile([128, 512], bf16, tag="t2")
                nc.vector.tensor_mul(t2, qs, sinq_sb[:, sp])
                nc.vector.tensor_add(t2, t1, t2)
                nc.vector.tensor_mul(qf[p], t2, ps_rqb[:])
                if DBG:
                    nc.sync.dma_start(out=dbg_qf[:, B, p, :], in_=qf[p])

            # ======== RoPE k (bf16)
            t1k = tmpp.tile([64, 512], bf16, tag="t1")
            nc.vector.tensor_mul(t1k, kc, cosk_sb[:, sp])
            ks = tmpp.tile([64, 512], bf16, tag="qs")
            nc.vector.tensor_copy(ks[0:32, :], kc[32:64, :])
            nc.vector.tensor_copy(ks[32:64, :], kc[0:32, :])
            t2k = tmpp.tile([64, 512], bf16, tag="t2")
            nc.vector.tensor_mul(t2k, ks, sink_sb[:, sp])
            nc.vector.tensor_add(kk2[0:64, sp], t1k, t2k)
            nc.vector.tensor_copy(kk2[64:128, sp], kk2[0:64, sp])

            # ======== gate pass ([4,512] in small bank)
            ps_g4 = psum.tile([34, 512], f32, tag="small", bufs=1, name="psg4")
            for hc in range(HC):
                nc.tensor.matmul(ps_g4[:], Wg_sb[:, hc, :], xsb[:, hc, sp],
                                 start=(hc == 0), stop=(hc == HC - 1))
            for p in range(2):
                nc.scalar.activation(eg_sb[:, B, p, :], ps_g4[32 * p:32 * p + 2, :],
                                     AF.Exp, scale=-1.0)
            if DBG:
                egf = smal.tile([2, 2, 512], f32, tag="egf")
                for p in range(2):
                    nc.vector.tensor_copy(egf[:, p, :], eg_sb[:, B, p, :])
                nc.sync.dma_start(out=dbg_eg[:, B, :, :], in_=egf)

            # ======== attention for si-block B
            at = [qfp.tile([128, 512], bf16, tag=f"at{p}", name=f"at{p}", bufs=2)
                  for p in range(2)]
            for p in range(2):
                ps_att = [psum.tile([128, 512], f32, tag="att", bufs=2,
                                    name=f"psatt{hh}") for hh in range(2)]
                for J in range(4 * B + 4):
                    off = max(0, (J - 4 * B) * 128)
                    ex = []
                    for hh in range(2):
                        rb = hh * 64
                        ps_s = psum.tile([128, 512], f32, tag="sc", bufs=2,
                                         name="pss")
                        nc.tensor.matmul(
                            ps_s[:, off:512],
                            kk2[rb:rb + 64, J * 128:(J + 1) * 128],
                            qf[p][rb:rb + 64, off:512],
                            start=True, stop=True,
                            tile_position=(rb, 0))
                        et = expp.tile([128, 512], bf16, tag="expT", bufs=5,
                                       name="et")
                        nc.scalar.activation(et[:, off:512], ps_s[:, off:512],
                                             AF.Exp, scale=rkT_sb[:, J:J + 1])
                        if off > 0 or J == 4 * B:
                            nc.gpsimd.tensor_mul(et[:, off:off + 128],
                                                 et[:, off:off + 128], tri)
                        ex.append(et)
                    for hh in range(2):
                        nc.tensor.matmul(
                            ps_att[hh][0:65, off:512],
                            v_sb[:, J, :],
                            ex[hh][:, off:512],
                            start=(J == 0), stop=(J == 4 * B + 3))

                # denominators -> scale s = sigmoid(gate)/den
                den2 = smal.tile([2, 512], f32, tag="smA")
                for hh in range(2):
                    dh = smal.tile([1, 512], f32, tag="smB")
                    nc.scalar.copy(dh, ps_att[hh][64:65, :])
                    nc.sync.dma_start(out=bass.AP(
                        tensor=den2.tensor, offset=den2[hh:hh + 1, :].offset,
                        ap=den2[hh:hh + 1, :].ap), in_=dh)
                if DBG:
                    nc.sync.dma_start(out=dbg_den[:, B, p, :], in_=den2)
                u_t = smal.tile([2, 512], f32, tag="den4")
                nc.vector.scalar_tensor_tensor(u_t, eg_sb[:, B, p, :], 1.0, den2,
                                               mybir.AluOpType.add,
                                               mybir.AluOpType.mult)
                s_t = smal.tile([2, 512], f32, tag="smB", name="s_t")
                nc.scalar.activation(s_t, u_t[:].bitcast(u32), AF.Exp,
                                     bias=b_rcp[0:2, :], scale=-EXPBIT_SCALE)
                for it in range(1):
                    tu = smal.tile([2, 512], f32, tag="smA", name="tu")
                    nc.vector.tensor_mul(tu, u_t, s_t)
                    nc.vector.tensor_scalar(tu, tu, -1.0, 2.0,
                                            mybir.AluOpType.mult,
                                            mybir.AluOpType.add)
                    s_n = smal.tile([2, 512], f32, tag="smB", name="s_n")
                    nc.vector.tensor_mul(s_n, s_t, tu)
                    s_t = s_n
                ps_sb = psum.tile([128, 512], f32, tag="small", bufs=1,
                                  name="pssb")
                nc.tensor.matmul(ps_sb[:], sel2[0:2, :], s_t, start=True, stop=True)
                atc = tmpp.tile([128, 512], f32, tag="atc")
                for hh in range(2):
                    rb = hh * 64
                    nc.vector.tensor_copy(atc[rb:rb + 64, :], ps_att[hh][0:64, :])
                nc.vector.tensor_mul(at[p], atc, ps_sb[:])
                if DBG:
                    nc.sync.dma_start(out=dbg_at[:, B, p, :], in_=at[p])

            st['at'] = at

        def emit_out(sib, st):
            B = sib
            at = st['at']
            # ======== output projection for this block's si-chunks
            for ss in range(4 * B, 4 * B + 4):
                ls = (ss - 4 * B) * 128
                for qtr in range(4):
                    ps_o = psum.tile([128, 512], f32, tag="qkv", bufs=3,
                                     name="pso")
                    nc.tensor.matmul(ps_o[:], at[0][:, ls:ls + 128],
                                     Wo_sb[:, 0, qtr * 512:(qtr + 1) * 512],
                                     start=True, stop=False)
                    nc.tensor.matmul(ps_o[:], at[1][:, ls:ls + 128],
                                     Wo_sb[:, 1, qtr * 512:(qtr + 1) * 512],
                                     start=False, stop=True)
                    ot = outs.tile([128, 512], f32, tag="ot")
                    nc.vector.tensor_copy(ot, ps_o[:])
                    nc.gpsimd.dma_start(
                        out=out_d[ss * 128:(ss + 1) * 128, qtr * 512:(qtr + 1) * 512],
                        in_=ot)

        st = {}
        st[0] = emit_qkv(0)
        st[1] = emit_qkv(1)
        for sib in range(NB):
            emit_att(sib, st[sib])
            if sib + 2 < NB:
                st[sib + 2] = emit_qkv(sib + 2)
            if sib >= 1:
                emit_out(sib - 1, st[sib - 1])
                del st[sib - 1]
        emit_out(NB - 1, st[NB - 1])

        if DBG:
            nc.sync.dma_start(out=dbg_kk2[:, :], in_=kk2)
            nc.sync.dma_start(out=dbg_v[:, :], in_=v_sb[:, :, :])
            nc.sync.dma_start(out=dbg_rkT[:, :], in_=rkT_sb)

    nc.compile()
    return nc


def _get_nc():
    if "nc" not in _BUILT:
        _BUILT["nc"] = _build_nc()
    return _BUILT["nc"]


# ---------------------------------------------------------------- entry point
def _install_ntff_hook():
    import types
    try:
        import antenv
        if "antenv.axon_hooks" in sys.modules:
            return True
        mod = types.ModuleType("antenv.axon_hooks")
        holder = [None]
        mod.set_axon_ntff_profile_hook = lambda h: holder.__setitem__(0, h)
        mod.get_axon_ntff_profile_hook = lambda: holder[0]
        sys.modules["antenv.axon_hooks"] = mod
        antenv.axon_hooks = mod
        from trn_agent_boot.trn_boot import _ntff_profile_via_ctypes
        hook = _ntff_profile_via_ctypes("/opt/axon/libaxon_pjrt.so")
        if hook is None:
            return False
        mod.set_axon_ntff_profile_hook(hook)
        return True
    except Exception:
        return False


def kernel(hidden_states, Wq, Wk, Wv, Wo, g_q, g_k):
    global LAST_EXEC_NS
    from concourse.bass_utils import run_bass_kernel_spmd

    in_maps = _host_prep(hidden_states, Wq, Wk, Wv, Wo, g_q, g_k)
    nc = _get_nc()
    trace = os.environ.get("KERNEL_TRACE", "0") == "1"
    if trace:
        trace = _install_ntff_hook()
    res = run_bass_kernel_spmd(nc, in_maps, list(range(NCORES)), trace=trace)
    LAST_EXEC_NS = res.exec_time_ns
    out = np.zeros((S, H), np.float32)
    for c in range(NCORES):
        out += res.results[c]["out"]
    return out.reshape(1, S, H).astype(np.float32)


# revision 27
# speedup vs baseline: 2.2140x; 1.1042x over previous
"""GatedAttention TRN2 kernel — 8-core tensor-parallel (1 kv-head group per core).

v1 restructure vs baseline: xT resident in SBUF (dense PE stream, HAM-warm),
fast PSUM->bf16 evacuation with downstream math in bf16 DVE 2x modes,
rotate-half copies on GpSimd, V transpose via DMA xbar, gate as separate
1-bank pass, fused [4,512] RMS-q chains, causal masks on GpSimd.

Per-core dataflow (feature-on-partition "T" layouts):
  qkvT = W_c.T @ xT           (bf16 matmuls, PSUM accumulation over 16 h-chunks)
  RMS scales via ones-selector matmuls (partition-dim sums), ln/exp on ACT
  RoPE on DVE with host-prefolded bf16 cos/sin tables
  scoresT[sj,si] per head, row-tiled head pairs on the PE array
  exp on ACT with per-partition scale = 0.125 * rsqrt(mean k^2)  (no max-sub:
  |scores*scale| <= 8 by Cauchy-Schwarz after RMS norm)
  P@V with V augmented by a ones column (M=65) -> fused softmax denominators
  out_partial = attnT_scaled.T @ Wo_c ; host sums the 8 partials.
"""
import math
import os
import sys
import numpy as np
import ml_dtypes

BF16 = ml_dtypes.bfloat16

H, NH, KVH, HD = 2048, 32, 8, 64
G = NH // KVH          # 4 q heads per core
S = 2048
EPS = 1e-6
THETA = 1000000.0
SCALE = 1.0 / math.sqrt(HD)
NCORES = 8
HC = H // 128          # 16 h-chunks
NB = S // 512          # 4 si-blocks
NJ = S // 128          # 16 sj-chunks

_BUILT = {}
LAST_EXEC_NS = None


# ---------------------------------------------------------------- host prep
def _host_prep(hidden_states, Wq, Wk, Wv, Wo, g_q, g_k):
    x = np.ascontiguousarray(np.asarray(hidden_states, np.float32).reshape(S, H))
    Wq = np.asarray(Wq, np.float32)
    Wk = np.asarray(Wk, np.float32)
    Wv = np.asarray(Wv, np.float32)
    Wo = np.asarray(Wo, np.float32)
    g_q = np.asarray(g_q, np.float32)
    g_k = np.asarray(g_k, np.float32)

    xT = np.ascontiguousarray(x.T).astype(BF16)

    inv_freq = 1.0 / (THETA ** (np.arange(0, HD, 2, dtype=np.float32) / HD))
    pos = np.arange(S, dtype=np.float32)
    emb = np.concatenate([pos[:, None] * inv_freq[None, :]] * 2, axis=-1)  # [S,64]
    cos = np.cos(emb).T.astype(np.float32)   # [64, S]
    sin = np.sin(emb).T.astype(np.float32)
    sign = np.where(np.arange(HD) < HD // 2, -1.0, 1.0).astype(np.float32)[:, None]
    cosq1 = cos * g_q[:, None]
    sinq1 = sin * sign * np.roll(g_q, -32)[:, None]
    # duplicate to 128 partitions (2 heads per p-pair)
    cosq = np.ascontiguousarray(np.concatenate([cosq1, cosq1], 0)).astype(BF16)
    sinq = np.ascontiguousarray(np.concatenate([sinq1, sinq1], 0)).astype(BF16)
    cosk = np.ascontiguousarray(cos * g_k[:, None]).astype(BF16)
    sink = np.ascontiguousarray(sin * sign * np.roll(g_k, -32)[:, None]).astype(BF16)

    in_maps = []
    for c in range(NCORES):
        Wq_g = Wq[:, c * (G * HD + G):(c + 1) * (G * HD + G)]
        W_c = np.ascontiguousarray(np.concatenate(
            [Wq_g[:, :G * HD],
             Wk[:, c * HD:(c + 1) * HD],
             Wv[:, c * HD:(c + 1) * HD]], axis=1))              # [H, 384]
        gpad = np.zeros((H, 34), np.float32)
        for p in range(2):
            for hh in range(2):
                gpad[:, 32 * p + hh] = Wq_g[:, G * HD + 2 * p + hh]
        Wg_c = np.ascontiguousarray(gpad)                        # [H, 34]
        Wo_c = np.ascontiguousarray(Wo[c * G * HD:(c + 1) * G * HD, :])  # [256,H]
        sel2 = np.zeros((34, 128), np.float32)
        for bp in (0, 32):
            sel2[bp, 0:64] = 1.0
            sel2[bp + 1, 64:128] = 1.0
        in_maps.append({"xT": xT, "W": W_c.astype(BF16), "Wg": Wg_c.astype(BF16),
                        "Wo": Wo_c.astype(BF16), "sel2": sel2,
                        "cosq": cosq, "sinq": sinq, "cosk": cosk, "sink": sink})
    return in_maps


# ---------------------------------------------------------------- bass build
def _build_nc():
    import concourse.bass as bass
    import concourse.mybir as mybir
    import concourse.tile as tile
    from concourse import bacc
    from concourse.masks import make_identity, make_upper_triangular

    dt = mybir.dt
    f32 = dt.float32
    bf16 = dt.bfloat16
    u32 = dt.uint32
    AF = mybir.ActivationFunctionType

    nc = bacc.Bacc("TRN2", target_bir_lowering=False, debug=False,
                   num_devices=NCORES)

    xT_d = nc.dram_tensor("xT", [H, S], bf16, kind="ExternalInput")
    W_d = nc.dram_tensor("W", [H, 384], bf16, kind="ExternalInput")
    Wg_d = nc.dram_tensor("Wg", [H, 34], bf16, kind="ExternalInput")
    Wo_d = nc.dram_tensor("Wo", [G * HD, H], bf16, kind="ExternalInput")
    cosq_d = nc.dram_tensor("cosq", [128, S], bf16, kind="ExternalInput")
    sinq_d = nc.dram_tensor("sinq", [128, S], bf16, kind="ExternalInput")
    cosk_d = nc.dram_tensor("cosk", [HD, S], bf16, kind="ExternalInput")
    sink_d = nc.dram_tensor("sink", [HD, S], bf16, kind="ExternalInput")
    sel2_d = nc.dram_tensor("sel2", [34, 128], f32, kind="ExternalInput")
    out_d = nc.dram_tensor("out", [S, H], f32, kind="ExternalOutput")

    DBG = os.environ.get("KERNEL_DEBUG", "0") == "1"
    if DBG:
        dbg_kk2 = nc.dram_tensor("dbg_kk2", [128, S], bf16, kind="ExternalOutput")
        dbg_v = nc.dram_tensor("dbg_v", [128, NJ * 65], bf16, kind="ExternalOutput")
        dbg_qf = nc.dram_tensor("dbg_qf", [128, NB, 2, 512], bf16,
                                kind="ExternalOutput")
        dbg_rq = nc.dram_tensor("dbg_rq", [4, NB, 512], f32, kind="ExternalOutput")
        dbg_rkT = nc.dram_tensor("dbg_rkT", [128, NJ], f32, kind="ExternalOutput")
        dbg_eg = nc.dram_tensor("dbg_eg", [2, NB, 2, 512], f32,
                                kind="ExternalOutput")
        dbg_at = nc.dram_tensor("dbg_at", [128, NB, 2, 512], bf16,
                                kind="ExternalOutput")
        dbg_den = nc.dram_tensor("dbg_den", [2, NB, 2, 512], f32,
                                 kind="ExternalOutput")

    def bcast_rows(src, reps):
        """src [r, n] -> AP iterating [r, reps, n] (row-replication)."""
        return bass.AP(tensor=src.tensor, offset=src.offset,
                       ap=[src.ap[0], [0, reps], src.ap[1]])

    import contextlib
    with tile.TileContext(nc) as tc, contextlib.ExitStack() as ctx:
        const = ctx.enter_context(tc.tile_pool(name="const", bufs=1))
        big = ctx.enter_context(tc.tile_pool(name="big", bufs=1))
        evp = ctx.enter_context(tc.tile_pool(name="evp", bufs=3))
        tmpp = ctx.enter_context(tc.tile_pool(name="tmp", bufs=2))
        qfp = ctx.enter_context(tc.tile_pool(name="qfp", bufs=3))
        expp = ctx.enter_context(tc.tile_pool(name="expp", bufs=5))
        outs = ctx.enter_context(tc.tile_pool(name="outs", bufs=4))
        smal = ctx.enter_context(tc.tile_pool(name="smal", bufs=2))
        bcp = ctx.enter_context(tc.tile_pool(name="bc", bufs=2))
        psum = ctx.enter_context(tc.tile_pool(name="ps", bufs=1, space="PSUM"))

        # ---------------- constants
        id64 = const.tile([64, 64], bf16, tag="id64")
        make_identity(nc, id64)
        tri = const.tile([128, 128], bf16, tag="tri")
        make_upper_triangular(nc, tri, val=1.0, diag=True)
        ones = const.tile([128, 1], bf16, tag="ones")
        nc.vector.memset(ones, 1.0)
        esel4 = const.tile([128, 2, 34], bf16, tag="esel4")
        nc.vector.memset(esel4, 0.0)
        for p in range(2):
            nc.vector.memset(esel4[0:64, p, 32 * p:32 * p + 1], 1.0)
            nc.vector.memset(esel4[64:128, p, 32 * p + 1:32 * p + 2], 1.0)
        sel2 = const.tile([34, 128], f32, tag="sel2")
        nc.sync.dma_start(out=sel2, in_=sel2_d[:, :])
        SIGMA = 0.0430
        EXPBIT_SCALE = math.log(2.0) / (1 << 23)
        b_rsq = const.tile([128, 1], f32, tag="brsq")
        nc.vector.memset(b_rsq, 0.5 * math.log(2.0) * (127 + SIGMA + 6))
        b_rcp = const.tile([128, 1], f32, tag="brcp")
        nc.vector.memset(b_rcp, math.log(2.0) * (127 + SIGMA))

        # ---------------- resident weights / tables
        W_sb = big.tile([128, HC, 384], bf16, tag="W")
        nc.sync.dma_start(out=W_sb, in_=W_d.ap().rearrange(
            "(hc p) c -> p hc c", p=128))
        Wg_sb = big.tile([128, HC, 34], bf16, tag="Wg")
        nc.sync.dma_start(out=Wg_sb, in_=Wg_d.ap().rearrange(
            "(hc p) c -> p hc c", p=128))
        Wo_sb = big.tile([128, 2, H], bf16, tag="Wo")
        nc.sync.dma_start(out=Wo_sb, in_=Wo_d.ap().rearrange(
            "(cc p) h -> p cc h", p=128))
        cosq_sb = big.tile([128, S], bf16, tag="cosq")
        nc.sync.dma_start(out=cosq_sb, in_=cosq_d[:, :])
        sinq_sb = big.tile([128, S], bf16, tag="sinq")
        nc.sync.dma_start(out=sinq_sb, in_=sinq_d[:, :])
        cosk_sb = big.tile([64, S], bf16, tag="cosk")
        nc.sync.dma_start(out=cosk_sb, in_=cosk_d[:, :])
        sink_sb = big.tile([64, S], bf16, tag="sink")
        nc.sync.dma_start(out=sink_sb, in_=sink_d[:, :])

        # resident x (feature-on-partition), 16 separate chunk tiles
        xsb = []
        for hc in range(HC):
            xc = big.tile([128, S], bf16, tag=f"xsb{hc}", name=f"xsb{hc}")
            eng = nc.gpsimd if hc % 2 == 0 else nc.sync
            eng.dma_start(out=xc, in_=xT_d[hc * 128:(hc + 1) * 128, :])
            xsb.append(xc)

        # ---------------- persistent activations
        kk2 = big.tile([128, S], bf16, tag="kk2")
        v_sb = big.tile([128, NJ, 65], bf16, tag="v")
        nc.vector.memset(v_sb[:, :, 64:65], 1.0)
        rkT_sb = big.tile([128, NJ], f32, tag="rkT")
        eg_sb = big.tile([2, NB, 2, 512], bf16, tag="eg")

        def emit_qkv(sib):
            B = sib
            sp = slice(sib * 512, (sib + 1) * 512)

            # ======== QKV projection, cc-outer (1 live bank + 1 evacuating)
            qc = [None, None]
            kc = vc = None
            for cc in range(3):
                ps_c = psum.tile([128, 512], f32, tag="qkv", bufs=2,
                                 name=f"pscc{cc}")
                for hc in range(HC):
                    nc.tensor.matmul(ps_c[:],
                                     W_sb[:, hc, cc * 128:(cc + 1) * 128],
                                     xsb[hc][:, sp],
                                     start=(hc == 0), stop=(hc == HC - 1))
                if cc < 2:
                    qc[cc] = evp.tile([128, 512], bf16, tag=f"qc{cc}",
                                      name=f"qc{cc}")
                    nc.vector.tensor_copy(qc[cc], ps_c[:])
                else:
                    kc = evp.tile([64, 512], bf16, tag="kc")
                    nc.vector.tensor_copy(kc, ps_c[0:64, :])
                    vc = evp.tile([64, 512], bf16, tag="vc")
                    nc.vector.tensor_copy(vc, ps_c[64:128, :])

            # ======== RMS q: fused [4,512] sums for both head pairs
            sq = [tmpp.tile([128, 512], bf16, tag=f"sq{p}", name=f"sq{p}")
                  for p in range(2)]
            for p in range(2):
                nc.vector.tensor_mul(sq[p], qc[p], qc[p])
            ps_rq = psum.tile([34, 512], f32, tag="att", bufs=2, name="psrq")
            for p in range(2):
                nc.tensor.matmul(ps_rq[:], esel4[:, p, :], sq[p],
                                 start=(p == 0), stop=(p == 1))
            rqs = smal.tile([34, 512], f32, tag="smC", name="rqs")
            nc.scalar.copy(rqs, ps_rq[:])
            y0 = smal.tile([34, 512], f32, tag="smB", name="y0q")
            nc.scalar.activation(y0, rqs.bitcast(u32), AF.Exp,
                                 bias=b_rsq[0:34, :], scale=-0.5 * EXPBIT_SCALE)
            for it in range(2):
                last = (it == 1)
                tn = smal.tile([34, 512], f32, tag="smA", name="tn")
                nc.vector.tensor_mul(tn, rqs, y0)
                nc.vector.tensor_mul(tn, tn, y0)
                nc.vector.tensor_scalar(tn, tn, -0.5 / HD, 1.5,
                                        mybir.AluOpType.mult, mybir.AluOpType.add)
                if last:
                    yn = smal.tile([34, 512], f32, tag="smBb", bufs=3, name="yn")
                else:
                    yn = smal.tile([34, 512], f32, tag="smB", name="yn")
                nc.vector.tensor_mul(yn, y0, tn)
                y0 = yn
            rq4 = y0                                            # [34,512] f32
            if DBG:
                for p in range(2):
                    nc.sync.dma_start(out=dbg_rq[2 * p:2 * p + 2, B, :],
                                      in_=rq4[32 * p:32 * p + 2, :])

            # ======== RMS k: [128,4] per-chunk sums
            ksq = tmpp.tile([64, 512], bf16, tag="ksq")
            nc.vector.tensor_mul(ksq, kc, kc)
            ps_rk = psum.tile([128, 4], f32, tag="att", bufs=2, name="psrk")
            for j in range(4):
                nc.tensor.matmul(ps_rk[:, j:j + 1],
                                 ksq[:, j * 128:(j + 1) * 128],
                                 ones[0:64, :], start=True, stop=True)
            rks = smal.tile([128, 4], f32, tag="smC", name="rks")
            nc.scalar.copy(rks, ps_rk[:])
            yk = smal.tile([128, 4], f32, tag="smB", name="yk")
            nc.scalar.activation(yk, rks.bitcast(u32), AF.Exp,
                                 bias=b_rsq, scale=-0.5 * EXPBIT_SCALE)
            for it in range(2):
                last = (it == 1)
                tk = smal.tile([128, 4], f32, tag="smA", name="tk")
                nc.vector.tensor_mul(tk, rks, yk)
                nc.vector.tensor_mul(tk, tk, yk)
                nc.vector.tensor_scalar(tk, tk,
                                        (-0.5 * SCALE / HD) if last else (-0.5 / HD),
                                        (1.5 * SCALE) if last else 1.5,
                                        mybir.AluOpType.mult, mybir.AluOpType.add)
                if last:
                    nc.vector.tensor_mul(rkT_sb[:, sib * 4:(sib + 1) * 4], yk, tk)
                else:
                    ykn = smal.tile([128, 4], f32, tag="smB", name="ykn")
                    nc.vector.tensor_mul(ykn, yk, tk)
                    yk = ykn

            # ======== V transpose (PE transpose, bf16)
            for j in range(4):
                J = sib * 4 + j
                ps_v = psum.tile([128, 64], bf16, tag="att", bufs=2,
                                 name="psv")
                nc.tensor.transpose(ps_v[:], vc[:, j * 128:(j + 1) * 128], id64)
                nc.vector.tensor_copy(v_sb[:, J, 0:64], ps_v[:])

            return {'qc': qc, 'kc': kc, 'rq4': rq4}

        def emit_att(sib, st, fillers):
            B = sib
            qc, kc, rq4 = st['qc'], st['kc'], st['rq4']
            sp = slice(sib * 512, (sib + 1) * 512)
            # ======== RoPE q (bf16)
            qf = [qfp.tile([128, 512], bf16, tag=f"qf{p}", name=f"qf{p}")
                  for p in range(2)]
            for p in range(2):
                ps_rqb = psum.tile([128, 512], f32, tag="att", bufs=2,
                                   name="psrqb")
                nc.tensor.matmul(ps_rqb[:], sel2[32 * p:32 * p + 2, :],
                                 rq4[32 * p:32 * p + 2, :],
                                 start=True, stop=True,
                                 tile_position=(32 * p, 0))
                t1 = tmpp.tile([128, 512], bf16, tag="t1")
                nc.vector.tensor_mul(t1, qc[p], cosq_sb[:, sp])
                qs = tmpp.tile([128, 512], bf16, tag="qs")
                for g in range(2):
                    b = g * 64
                    nc.vector.tensor_copy(qs[b:b + 32, :], qc[p][b + 32:b + 64, :])
                    nc.vector.tensor_copy(qs[b + 32:b + 64, :], qc[p][b:b + 32, :])
                t2 = tmpp.tile([128, 512], bf16, tag="t2")
                nc.vector.tensor_mul(t2, qs, sinq_sb[:, sp])
                nc.vector.tensor_add(t2, t1, t2)
                nc.vector.tensor_mul(qf[p], t2, ps_rqb[:])
                if DBG:
                    nc.sync.dma_start(out=dbg_qf[:, B, p, :], in_=qf[p])

            # ======== RoPE k (bf16)
            t1k = tmpp.tile([64, 512], bf16, tag="t1")
            nc.vector.tensor_mul(t1k, kc, cosk_sb[:, sp])
            ks = tmpp.tile([64, 512], bf16, tag="qs")
            nc.vector.tensor_copy(ks[0:32, :], kc[32:64, :])
            nc.vector.tensor_copy(ks[32:64, :], kc[0:32, :])
            t2k = tmpp.tile([64, 512], bf16, tag="t2")
            nc.vector.tensor_mul(t2k, ks, sink_sb[:, sp])
            nc.vector.tensor_add(kk2[0:64, sp], t1k, t2k)
            nc.vector.tensor_copy(kk2[64:128, sp], kk2[0:64, sp])

            # ======== gate pass ([4,512] in small bank)
            ps_g4 = psum.tile([34, 512], f32, tag="att", bufs=2, name="psg4")
            for hc in range(HC):
                nc.tensor.matmul(ps_g4[:], Wg_sb[:, hc, :], xsb[hc][:, sp],
                                 start=(hc == 0), stop=(hc == HC - 1))
            for p in range(2):
                nc.scalar.activation(eg_sb[:, B, p, :], ps_g4[32 * p:32 * p + 2, :],
                                     AF.Exp, scale=-1.0)
            if DBG:
                egf = smal.tile([2, 2, 512], f32, tag="egf")
                for p in range(2):
                    nc.vector.tensor_copy(egf[:, p, :], eg_sb[:, B, p, :])
                nc.sync.dma_start(out=dbg_eg[:, B, :, :], in_=egf)

            # ======== attention for si-block B
            at = [qfp.tile([128, 512], bf16, tag=f"at{p}", name=f"at{p}", bufs=2)
                  for p in range(2)]
            for p in range(2):
                ps_att = [psum.tile([128, 512], f32, tag="att", bufs=2,
                                    name=f"psatt{hh}") for hh in range(2)]
                for J in range(4 * B + 4):
                    off = max(0, (J - 4 * B) * 128)
                    ps_s2 = psum.tile([128, 2, 512], f32, tag="sc", bufs=2,
                                      name="pss")
                    for hh in range(2):
                        rb = hh * 64
                        nc.tensor.matmul(
                            ps_s2[:, hh, off:512],
                            kk2[rb:rb + 64, J * 128:(J + 1) * 128],
                            qf[p][rb:rb + 64, off:512],
                            start=True, stop=True,
                            tile_position=(rb, 0))
                    et2 = expp.tile([128, 2, 512], bf16, tag="expT", bufs=3,
                                    name="et")
                    nc.scalar.activation(et2[:, :, off:512], ps_s2[:, :, off:512],
                                         AF.Exp, scale=rkT_sb[:, J:J + 1])
                    if off > 0 or J == 4 * B:
                        for hh in range(2):
                            nc.gpsimd.tensor_mul(et2[:, hh, off:off + 128],
                                                 et2[:, hh, off:off + 128], tri)
                    for hh in range(2):
                        nc.tensor.matmul(
                            ps_att[hh][0:65, off:512],
                            v_sb[:, J, :],
                            et2[:, hh, off:512],
                            start=(J == 0), stop=(J == 4 * B + 3))
                    if fillers:
                        fillers.pop(0)()

                # denominators -> scale s = sigmoid(gate)/den
                den2 = smal.tile([2, 512], f32, tag="smA")
                for hh in range(2):
                    dh = smal.tile([1, 512], f32, tag="smB")
                    nc.vector.tensor_copy(dh, ps_att[hh][64:65, :])
                    nc.sync.dma_start(out=bass.AP(
                        tensor=den2.tensor, offset=den2[hh:hh + 1, :].offset,
                        ap=den2[hh:hh + 1, :].ap), in_=dh)
                if DBG:
                    nc.sync.dma_start(out=dbg_den[:, B, p, :], in_=den2)
                u_t = smal.tile([2, 512], f32, tag="den4")
                nc.vector.scalar_tensor_tensor(u_t, eg_sb[:, B, p, :], 1.0, den2,
                                               mybir.AluOpType.add,
                                               mybir.AluOpType.mult)
                s_t = smal.tile([2, 512], f32, tag="smB", name="s_t")
                nc.scalar.activation(s_t, u_t[:].bitcast(u32), AF.Exp,
                                     bias=b_rcp[0:2, :], scale=-EXPBIT_SCALE)
                for it in range(1):
                    tu = smal.tile([2, 512], f32, tag="smA", name="tu")
                    nc.vector.tensor_mul(tu, u_t, s_t)
                    nc.vector.tensor_scalar(tu, tu, -1.0, 2.0,
                                            mybir.AluOpType.mult,
                                            mybir.AluOpType.add)
                    s_n = smal.tile([2, 512], f32, tag="smB", name="s_n")
                    nc.vector.tensor_mul(s_n, s_t, tu)
                    s_t = s_n
                ps_sb = psum.tile([128, 512], f32, tag="att", bufs=2,
                                  name="pssb")
                nc.tensor.matmul(ps_sb[:], sel2[0:2, :], s_t, start=True, stop=True)
                atc = tmpp.tile([128, 512], f32, tag="atc")
                for hh in range(2):
                    rb = hh * 64
                    nc.vector.tensor_copy(atc[rb:rb + 64, :], ps_att[hh][0:64, :])
                nc.vector.tensor_mul(at[p], atc, ps_sb[:])
                if DBG:
                    nc.sync.dma_start(out=dbg_at[:, B, p, :], in_=at[p])

            st['at'] = at

        def make_out_fillers(sib, st):
            at = st['at']
            fl = []
            for ss in range(4 * sib, 4 * sib + 4):
                ls = (ss - 4 * sib) * 128
                for qtr in range(4):
                    def one(ss=ss, ls=ls, qtr=qtr):
                        ps_o = psum.tile([128, 512], f32, tag="qkv", bufs=2,
                                         name="pso")
                        nc.tensor.matmul(ps_o[:], at[0][:, ls:ls + 128],
                                         Wo_sb[:, 0, qtr * 512:(qtr + 1) * 512],
                                         start=True, stop=False)
                        nc.tensor.matmul(ps_o[:], at[1][:, ls:ls + 128],
                                         Wo_sb[:, 1, qtr * 512:(qtr + 1) * 512],
                                         start=False, stop=True)
                        ot = outs.tile([128, 512], f32, tag="ot")
                        nc.vector.tensor_copy(ot, ps_o[:])
                        nc.gpsimd.dma_start(
                            out=out_d[ss * 128:(ss + 1) * 128,
                                      qtr * 512:(qtr + 1) * 512],
                            in_=ot)
                    fl.append(one)
            return fl

        st = {}
        st[0] = emit_qkv(0)
        st[1] = emit_qkv(1)
        fillers = []
        for sib in range(NB):
            emit_att(sib, st[sib], fillers)
            while fillers:
                fillers.pop(0)()
            if sib + 2 < NB:
                st[sib + 2] = emit_qkv(sib + 2)
            fillers = make_out_fillers(sib, st[sib])
        while fillers:
            fillers.pop(0)()

        if DBG:
            nc.sync.dma_start(out=dbg_kk2[:, :], in_=kk2)
            nc.sync.dma_start(out=dbg_v[:, :], in_=v_sb[:, :, :])
            nc.sync.dma_start(out=dbg_rkT[:, :], in_=rkT_sb)

    nc.compile()
    return nc


def _get_nc():
    if "nc" not in _BUILT:
        _BUILT["nc"] = _build_nc()
    return _BUILT["nc"]


# ---------------------------------------------------------------- entry point
def _install_ntff_hook():
    import types
    try:
        import antenv
        if "antenv.axon_hooks" in sys.modules:
            return True
        mod = types.ModuleType("antenv.axon_hooks")
        holder = [None]
        mod.set_axon_ntff_profile_hook = lambda h: holder.__setitem__(0, h)
        mod.get_axon_ntff_profile_hook = lambda: holder[0]
        sys.modules["antenv.axon_hooks"] = mod
        antenv.axon_hooks = mod
        from trn_agent_boot.trn_boot import _ntff_profile_via_ctypes
        hook = _ntff_profile_via_ctypes("/opt/axon/libaxon_pjrt.so")
        if hook is None:
            return False
        mod.set_axon_ntff_profile_hook(hook)
        return True
    except Exception:
        return False


def kernel(hidden_states, Wq, Wk, Wv, Wo, g_q, g_k):
    global LAST_EXEC_NS
    from concourse.bass_utils import run_bass_kernel_spmd

    in_maps = _host_prep(hidden_states, Wq, Wk, Wv, Wo, g_q, g_k)
    nc = _get_nc()
    trace = os.environ.get("KERNEL_TRACE", "0") == "1"
    if trace:
        trace = _install_ntff_hook()
    res = run_bass_kernel_spmd(nc, in_maps, list(range(NCORES)), trace=trace)
    LAST_EXEC_NS = res.exec_time_ns
    out = np.zeros((S, H), np.float32)
    for c in range(NCORES):
        out += res.results[c]["out"]
    return out.reshape(1, S, H).astype(np.float32)
